# revision 1
# baseline (speedup 1.0000x reference)
"""EGConv + GraphNorm + ReLU Trainium2 kernel (8 NeuronCores, SPMD).

Strategy (hardcoded for N=100000, E=3200000, D=128, H=8, B=4, A=['sum','max'],
G=64 graphs):
  - Nodes partitioned across 8 cores at graph boundaries (GraphNorm stays
    core-local). Each core owns its dst nodes and their incident edges.
  - Edges gathered per-dst via SWDGE dma_gather (int16 indices -> the padded
    global bases table is split into 4 chunks of 2*NMAX <= 32768 rows; the
    gather source AP per call is one chunk).
  - Per-core dst nodes are sorted by their per-chunk in-degree vector so that
    128-node tiles have tight per-chunk max degrees (less padding).
  - The gather table holds bases + BIG (BIG=512) for real rows and 0 for pad
    rows; padding slots point at pad rows, so
        segment_sum = reduce_sum - k_dst*BIG,   segment_max = reduce_max - BIG.
  - comb/bases matmuls, GraphNorm segment stats (via indicator matmuls) and
    the per-graph affine run on TensorE; reductions and the (comb x aggr)
    einsum run on VectorE/GpSimd.
"""

import math
import os
import numpy as np

# ---------------- problem constants (hardcoded per spec) ----------------
N = 100000
E = 3200000
D = 128
H = 8
NB = 4          # num bases
FH = 16         # per-head dim
G = 64          # graphs
EPS = 1e-5
P = 128
NCORES = 8
BF = 64         # bases feature dim = NB*FH
BIG = 512.0
GPAD = 16       # padded per-core graph count
COLCAP = 8      # max gather columns per call (8*128 = 1024 descriptors)
SLOTCAP = 128   # max T*Wtot slots per supertile (SBUF budget)
TMAX = 1

_F32 = np.float32


def _ceil(a, b):
    return -(-a // b) * b


# ======================================================================
# host preprocessing
# ======================================================================
def _prep(edge_index, batch_ptr):
    counts = np.bincount(batch_ptr, minlength=G).astype(np.int64)
    gcum = np.concatenate([[0], np.cumsum(counts)])  # [G+1]

    # core boundaries at graph boundaries, close to N/8 multiples
    gb = [0]
    for c in range(1, NCORES):
        tgt = N * c / NCORES
        g = int(np.argmin(np.abs(gcum - tgt)))
        g = min(max(g, gb[-1]), G - (NCORES - c))
        gb.append(g)
    gb.append(G)
    node0 = np.array([gcum[gb[c]] for c in range(NCORES)], np.int64)
    ncs = np.array([gcum[gb[c + 1]] - gcum[gb[c]] for c in range(NCORES)],
                   np.int64)
    NMAX = _ceil(int(ncs.max()) + 1, P)
    assert 2 * NMAX <= 32768, (NMAX, ncs.max())
    CHUNK = 2 * NMAX
    ntiles = NMAX // P

    src_g = np.asarray(edge_index[0], np.int64)
    dst_g = np.asarray(edge_index[1], np.int64)
    bounds = np.concatenate([node0, [N]])
    node_core = np.searchsorted(bounds, np.arange(N), side="right") - 1
    node_local = np.arange(N) - node0[node_core]

    src_core = node_core[src_g]
    dst_core = node_core[dst_g]

    cores = []
    for c in range(NCORES):
        n_c = int(ncs[c])
        m = dst_core == c
        es = src_g[m]
        ed = dst_g[m] - node0[c]
        # self loops
        own = np.arange(n_c, dtype=np.int64)
        es = np.concatenate([es, own + node0[c]])
        ed = np.concatenate([ed, own])
        ch = node_core[es] >> 1  # chunk of each edge's src

        # per-(dst, chunk) counts
        kmat = np.bincount(ed * 4 + ch, minlength=n_c * 4).reshape(n_c, 4)
        # sort desc by (max_c k_c, k0, k1, k2, k3) — tight per-tile windows
        kmx = kmat.max(axis=1)
        order = np.lexsort((-kmat[:, 3], -kmat[:, 2], -kmat[:, 1],
                            -kmat[:, 0], -kmx))
        pos_of = np.empty(n_c, np.int64)
        pos_of[order] = np.arange(n_c)
        cores.append(dict(n=n_c, node0=int(node0[c]), perm=order,
                          pos_of=pos_of, es=es, ed=ed, ch=ch, kmat=kmat,
                          g0=gb[c], g1=gb[c + 1]))

    # pid of a global node id: core*NMAX + sorted position
    pid_of = np.empty(N, np.int64)
    for c in range(NCORES):
        cc = cores[c]
        pid_of[cc["node0"]:cc["node0"] + cc["n"]] = c * NMAX + cc["pos_of"]

    # per-core sorted-space per-chunk CSR + k arrays
    for c in range(NCORES):
        cc = cores[c]
        dpos = cc["pos_of"][cc["ed"]]
        key = dpos * 4 + cc["ch"]
        eorder = np.argsort(key, kind="stable")
        rel = (pid_of[cc["es"]] - cc["ch"] * CHUNK)[eorder]
        assert rel.min() >= 0 and rel.max() < CHUNK
        kflat = np.bincount(key, minlength=NMAX * 4)
        indptr = np.concatenate([[0], np.cumsum(kflat)])
        ks = kflat.reshape(NMAX, 4)  # sorted space, dummies are 0
        cc["csr_data"] = rel.astype(np.int64)
        cc["csr_ptr"] = indptr
        cc["ks"] = ks

    # zero-row (pad row) per chunk: core 2c's first pad row
    zrow_rel = np.array([cores[2 * c]["n"] for c in range(4)], np.int64)

    # shared per-tile per-chunk window widths (max over cores)
    Wct = np.zeros((ntiles, 4), np.int64)
    for c in range(NCORES):
        km = cores[c]["ks"].reshape(ntiles, P, 4)
        Wct = np.maximum(Wct, km.max(axis=1))

    # supertile schedule
    sched = []
    t = 0
    while t < ntiles:
        T = 1
        while T < TMAX and t + T < ntiles:
            wc = Wct[t:t + T + 1].max(axis=0)
            if (T + 1) * int(wc.sum()) > SLOTCAP:
                break
            T += 1
        wc = Wct[t:t + T].max(axis=0)
        wtot = int(wc.sum())
        if wtot == 0:
            wtot = 1  # degenerate; shouldn't happen (self loops)
        coff = np.concatenate([[0], np.cumsum(wc)])
        calls = []  # (chunk, tile_rel, col0_in_msg, ncols, s16_off)
        s16 = 0
        for tr in range(T):
            for chx in range(4):
                w = int(wc[chx])
                done = 0
                while done < w:
                    nc_ = min(COLCAP, w - done)
                    calls.append((chx, tr, int(coff[chx]) + done, nc_, s16))
                    s16 += nc_ * 8
                    done += nc_
        sched.append(dict(t0=t, T=T, wc=wc.copy(), wtot=wtot,
                          coff=coff.copy(), calls=calls, s16=s16))
        t += T
    S16TOT = sum(s["s16"] for s in sched)

    # per-core device input arrays
    for c in range(NCORES):
        cc = cores[c]
        data, ptr, ks = cc["csr_data"], cc["csr_ptr"], cc["ks"]
        idx16 = np.zeros((P, S16TOT), np.int16)
        s16base = 0
        for s in sched:
            for (chx, tr, col0, ncols, s16off) in s["calls"]:
                tt = s["t0"] + tr
                dp = tt * P + np.arange(P)
                cnt = ks[dp, chx]
                start = ptr[dp * 4 + chx]
                j0 = col0 - int(s["coff"][chx])
                jj = j0 + np.arange(ncols)[None, :]
                gidx = np.minimum(start[:, None] + jj,
                                  max(len(data) - 1, 0))
                vals = data[gidx] if len(data) else np.zeros((P, ncols),
                                                             np.int64)
                mat = np.where(jj < cnt[:, None], vals, zrow_rel[chx])
                flat = mat.T.reshape(-1)  # [ncols*128], i = col*128 + p
                wrapped = flat.reshape(-1, 16).T.astype(np.int16)  # [16, S]
                idx16[:, s16base + s16off:
                      s16base + s16off + ncols * 8] = np.tile(wrapped, (8, 1))
            s16base += s["s16"]
        cc["idx16"] = idx16

        ktot = ks.sum(axis=1).astype(_F32)  # [NMAX] sorted space
        cc["kbig"] = (ktot.reshape(ntiles, P).T * -BIG).astype(_F32)  # [P,nt] (negated)
        real = (np.arange(NMAX) < cc["n"])
        cc["shift"] = np.where(real.reshape(ntiles, P).T, _F32(BIG),
                               _F32(0.0)).astype(_F32)

        # graph id per sorted position
        gid = np.full(NMAX, -1, np.int64)
        gnode = np.searchsorted(gcum, cc["node0"] + cc["perm"],
                                side="right") - 1 - cc["g0"]
        gid[:cc["n"]] = gnode
        ind = np.zeros((P, ntiles * GPAD), _F32)
        indT = np.zeros((GPAD, ntiles * P), _F32)
        for tt in range(ntiles):
            gl = gid[tt * P:(tt + 1) * P]
            valid = gl >= 0
            pidx = np.arange(P)[valid]
            gv = gl[valid]
            ind[pidx, tt * GPAD + gv] = 1.0
            indT[gv, tt * P + pidx] = 1.0
        cc["ind"] = ind
        cc["indT"] = indT
        cc["gid"] = gid

        cnt_loc = counts[cc["g0"]:cc["g1"]].astype(_F32)
        cntinv = np.zeros(GPAD, _F32)
        cntinv[:len(cnt_loc)] = 1.0 / np.maximum(cnt_loc, 1.0)
        cc["cntinv"] = cntinv

    return dict(cores=cores, NMAX=NMAX, CHUNK=CHUNK, ntiles=ntiles,
                sched=sched, S16TOT=S16TOT, zrow_rel=zrow_rel,
                node0=node0, ncs=ncs)


def _make_inputs(cfg, node, W_bases, W_comb, b_comb, bias_out, gn_weight,
                 gn_bias, gn_mean_scale):
    node = np.asarray(node, _F32)
    NMAX, ntiles = cfg["NMAX"], cfg["ntiles"]
    wcat = np.concatenate([np.asarray(W_bases, _F32),
                           np.asarray(W_comb, _F32)], axis=1)  # [128,128]
    bcomb = np.asarray(b_comb, _F32).reshape(1, BF)
    gaux = np.zeros((GPAD, 520), _F32)
    gaux[:, 1:129] = np.asarray(bias_out, _F32)[None, :]
    gaux[:, 129:257] = np.asarray(gn_mean_scale, _F32)[None, :]
    gaux[:, 257:385] = np.asarray(gn_weight, _F32)[None, :]
    gaux[:, 385:513] = np.asarray(gn_bias, _F32)[None, :]

    in_maps = []
    for c in range(NCORES):
        cc = cfg["cores"][c]
        nperm = np.zeros((NMAX, D), _F32)
        nperm[:cc["n"]] = node[cc["node0"]:cc["node0"] + cc["n"]][cc["perm"]]
        ga = gaux.copy()
        ga[:, 0] = cc["cntinv"]
        in_maps.append({
            "nodeT": np.ascontiguousarray(nperm.T),        # [128, NMAX]
            "wcat": wcat,
            "bcomb": bcomb,
            "idx": cc["idx16"],                            # [128, S16TOT]
            "kbig": np.ascontiguousarray(cc["kbig"]),      # [128, ntiles]
            "shift": np.ascontiguousarray(cc["shift"]),    # [128, ntiles]
            "ind": np.ascontiguousarray(
                cc["ind"].astype(np.dtype("bfloat16")
                                 if False else _F32)),     # [128, nt*16]
            "indT": np.ascontiguousarray(cc["indT"]),      # [16, nt*128]
            "gaux": ga,                                    # [16, 520]
        })
    return in_maps


# ======================================================================
# numpy simulation of the device algorithm (bit-approximate, for testing)
# ======================================================================
def _numpy_sim(cfg, in_maps):
    NMAX, CHUNK, ntiles = cfg["NMAX"], cfg["CHUNK"], cfg["ntiles"]
    # phase A+B: bases table (shared), per-core comb
    table = np.zeros((NCORES * NMAX, BF), _F32)
    combs = []
    for c in range(NCORES):
        im = in_maps[c]
        nodeT = im["nodeT"]
        full = nodeT.T @ im["wcat"]  # [NMAX, 128]
        bases = full[:, :BF]
        comb = full[:, BF:] + im["bcomb"][0][None, :]
        shift = im["shift"].T.reshape(-1)  # [NMAX]
        table[c * NMAX:(c + 1) * NMAX] = bases + shift[:, None]
        combs.append(comb)

    outs = []
    for c in range(NCORES):
        im = in_maps[c]
        h0 = np.zeros((NMAX, D), _F32)
        s16base = 0
        kbig = im["kbig"].T  # [ntiles, 128]
        for s in cfg["sched"]:
            T, wtot = s["T"], s["wtot"]
            msg = np.zeros((P, T, wtot, BF), _F32)
            for (chx, tr, col0, ncols, s16off) in s["calls"]:
                blk = im["idx"][:16, s16base + s16off:
                                s16base + s16off + ncols * 8]
                # unwrap: value[i] = blk[i % 16, i // 16]
                f2 = blk.T.reshape(-1)  # order (s, p): f2[s*16+p] = blk[p, s]
                vals = f2[:ncols * 128].astype(np.int64)
                rows = table[chx * CHUNK + vals.reshape(ncols, P)]
                msg[:, tr, col0:col0 + ncols, :] = rows.transpose(1, 0, 2)
            s16base += s["s16"]
            ssum = msg.sum(axis=2)                    # [P, T, 64]
            smax = msg.max(axis=2) - _F32(BIG)        # [P, T, 64]
            for tr in range(s["T"]):
                tt = s["t0"] + tr
                su = ssum[:, tr, :] + kbig[tt][:, None]
                aggcat = np.concatenate([su, smax[:, tr, :]], axis=1)
                comb = combs[c][tt * P:(tt + 1) * P]  # [128, 64]
                prod = (comb.reshape(P, H, 8, 1) *
                        aggcat.reshape(P, 1, 8, FH))
                h0[tt * P:(tt + 1) * P] = prod.sum(axis=2).reshape(P, D)
        # graphnorm
        ind = im["ind"].reshape(P, ntiles, GPAD)
        ga = im["gaux"]
        cntinv = ga[:, 0:1]
        bias_o = ga[:, 1:129]
        ms = ga[:, 129:257]
        gnw = ga[:, 257:385]
        gnb = ga[:, 385:513]
        s1 = np.zeros((GPAD, D), _F32)
        s2 = np.zeros((GPAD, D), _F32)
        for tt in range(ntiles):
            ht = h0[tt * P:(tt + 1) * P]
            s1 += ind[:, tt, :].T @ ht
            s2 += ind[:, tt, :].T @ (ht * ht)
        m0 = s1 * cntinv
        mh = m0 + bias_o
        e2 = s2 * cntinv + bias_o * (2 * m0 + bias_o)
        c0 = mh * ms
        var = e2 - 2 * c0 * mh + c0 * c0
        rstd = 1.0 / np.sqrt(var + EPS)
        Pm = gnw * rstd
        Qm = (bias_o - c0) * Pm + gnb
        indT = im["indT"].reshape(GPAD, ntiles, P)
        hfin = np.zeros((NMAX, D), _F32)
        for tt in range(ntiles):
            Pn = indT[:, tt, :].T @ Pm
            Qn = indT[:, tt, :].T @ Qm
            hfin[tt * P:(tt + 1) * P] = np.maximum(
                h0[tt * P:(tt + 1) * P] * Pn + Qn, 0.0)
        outs.append(hfin)
    return outs


def _assemble(cfg, per_core_h):
    out = np.zeros((N, D), _F32)
    for c in range(NCORES):
        cc = cfg["cores"][c]
        out[cc["node0"] + cc["perm"]] = per_core_h[c][:cc["n"]]
    return out


# ======================================================================
# device program
# ======================================================================
def _build(cfg):
    import concourse.bacc as bacc
    import concourse.tile as tile
    from concourse import mybir

    NMAX, CHUNK, ntiles = cfg["NMAX"], cfg["CHUNK"], cfg["ntiles"]
    S16TOT = cfg["S16TOT"]
    f32 = mybir.dt.float32
    bf16 = mybir.dt.bfloat16
    ALU = mybir.AluOpType
    ACT = mybir.ActivationFunctionType
    AX = mybir.AxisListType

    nc = bacc.Bacc("TRN2", target_bir_lowering=False, debug=False,
                   num_devices=NCORES, num_swdge_queues=4)

    nodeT = nc.dram_tensor("nodeT", [P, NMAX], f32, kind="ExternalInput").ap()
    wcat = nc.dram_tensor("wcat", [D, D], f32, kind="ExternalInput").ap()
    bcomb = nc.dram_tensor("bcomb", [1, BF], f32, kind="ExternalInput").ap()
    idx = nc.dram_tensor("idx", [P, S16TOT], mybir.dt.int16,
                         kind="ExternalInput").ap()
    kbig = nc.dram_tensor("kbig", [P, ntiles], f32, kind="ExternalInput").ap()
    shift = nc.dram_tensor("shift", [P, ntiles], f32,
                           kind="ExternalInput").ap()
    ind = nc.dram_tensor("ind", [P, ntiles * GPAD], f32,
                         kind="ExternalInput").ap()
    indT = nc.dram_tensor("indT", [GPAD, ntiles * P], f32,
                          kind="ExternalInput").ap()
    gaux = nc.dram_tensor("gaux", [GPAD, 520], f32, kind="ExternalInput").ap()
    h_out = nc.dram_tensor("h", [NMAX, D], f32, kind="ExternalOutput").ap()

    with tile.TileContext(nc) as tc:
        with (
            tc.tile_pool(name="dram", bufs=1, space="DRAM") as dram,
            tc.tile_pool(name="persist", bufs=1) as pp,
            tc.tile_pool(name="work", bufs=3) as wp,
            tc.tile_pool(name="idxp", bufs=5) as ixp,
            tc.tile_pool(name="msgp", bufs=3) as mp,
            tc.tile_pool(name="psum", bufs=2, space="PSUM") as psp,
            tc.tile_pool(name="statps", bufs=1, space="PSUM") as stp,
        ):
            bases_slice = dram.tile([NMAX, BF], f32)
            bases_full = dram.tile([NCORES * NMAX, BF], f32)

            # ---- constants / persistent
            wcat_s = pp.tile([D, D], f32)
            nc.sync.dma_start(wcat_s[:], wcat[:])
            bcomb_s = pp.tile([1, BF], f32)
            nc.sync.dma_start(bcomb_s[:], bcomb[:])
            ones1 = pp.tile([1, P], f32)
            nc.vector.memset(ones1[:], 1.0)
            negbig = pp.tile([P, 1], f32)
            nc.vector.memset(negbig[:], -BIG)
            kbig_s = pp.tile([P, ntiles], f32)
            nc.sync.dma_start(kbig_s[:], kbig[:])
            shift_s = pp.tile([P, ntiles], f32)
            nc.sync.dma_start(shift_s[:], shift[:])
            ind_s = pp.tile([P, ntiles * GPAD], f32)
            nc.sync.dma_start(ind_s[:], ind[:])
            gaux_s = pp.tile([GPAD, 520], f32)
            nc.sync.dma_start(gaux_s[:], gaux[:])

            comb_all = pp.tile([P, ntiles * BF], f32)
            h0_all = pp.tile([P, ntiles * D], f32)

            # ---------------- phase A: bases + comb ----------------
            ACHUNK = 10
            for t in range(ntiles):
                if t % ACHUNK == 0:
                    nblk = wp.tile([P, ACHUNK * P], f32, tag="nblk")
                    nb = min(ACHUNK, ntiles - t)
                    nc.sync.dma_start(nblk[:, :nb * P],
                                      nodeT[:, t * P:(t + nb) * P])
                nt = nblk[:, (t % ACHUNK) * P:(t % ACHUNK + 1) * P]
                ps = psp.tile([P, D], f32, tag="psA")
                nc.tensor.matmul(ps[:], nt, wcat_s[:], start=True,
                                 stop=False)
                nc.tensor.matmul(ps[:, BF:], ones1[:], bcomb_s[:],
                                 start=False, stop=True)
                bsh = wp.tile([P, BF], f32, tag="bsh")
                nc.scalar.activation(bsh[:], ps[:, :BF], ACT.Identity,
                                     bias=shift_s[:, t:t + 1], scale=1.0)
                nc.scalar.copy(comb_all[:, t * BF:(t + 1) * BF],
                               ps[:, BF:])
                nc.sync.dma_start(bases_slice[t * P:(t + 1) * P, :], bsh[:])

            # ---------------- phase B: allgather ----------------
            nc.gpsimd.collective_compute(
                "AllGather", ALU.bypass,
                replica_groups=[list(range(NCORES))],
                ins=[bases_slice.opt()],
                outs=[bases_full.opt()],
            )

            # ---------------- phase C: gather + aggregate + einsum ----
            stats = stp.tile([GPAD, 2 * D], f32)
            qrot = 0
            s16base = 0
            first_mm = True
            for si, s in enumerate(cfg["sched"]):
                assert s["T"] == 1
                tt = s["t0"]
                wtot = s["wtot"]
                idxt = ixp.tile([P, s["s16"]], mybir.dt.int16, tag="idxt")
                nc.sync.dma_start(idxt[:],
                                  idx[:, s16base:s16base + s["s16"]])
                msg = mp.tile([P, wtot, BF], f32, tag="msg")
                for (chx, tr, col0, ncols, s16off) in s["calls"]:
                    nc.gpsimd.dma_gather(
                        msg[:, col0:col0 + ncols, :],
                        bases_full[chx * CHUNK:(chx + 1) * CHUNK, :],
                        idxt[:, s16off:s16off + ncols * 8],
                        ncols * P, ncols * P, BF,
                        queue_num=qrot % 4,
                        single_packet=(ncols * P <= 1024),
                    )
                    qrot += 1
                aggcat = wp.tile([P, 2 * BF], f32, tag="aggcat")
                mv = msg[:].rearrange("p w f -> p f w")
                nc.vector.tensor_reduce(aggcat[:, :BF], mv, axis=AX.X,
                                        op=ALU.add)
                nc.vector.tensor_reduce(aggcat[:, BF:], mv, axis=AX.X,
                                        op=ALU.max)
                # corrections on ScalarE: sum += (-k*BIG) ; max += (-BIG)
                nc.scalar.activation(aggcat[:, :BF], aggcat[:, :BF],
                                     ACT.Identity,
                                     bias=kbig_s[:, tt:tt + 1], scale=1.0)
                nc.scalar.activation(aggcat[:, BF:], aggcat[:, BF:],
                                     ACT.Identity, bias=negbig[:], scale=1.0)
                prod = wp.tile([P, H, 8, FH], f32, tag="prod")
                cview = comb_all[:, tt * BF:(tt + 1) * BF].rearrange(
                    "p (h k) -> p h k", h=H)
                nc.vector.tensor_tensor(
                    out=prod[:],
                    in0=cview.to_broadcast([P, H, 8, FH]),
                    in1=aggcat[:].rearrange("p (k f) -> p k f", k=8)
                    [:, None, :, :].broadcast_to([P, H, 8, FH]),
                    op=ALU.mult)
                nc.vector.tensor_reduce(
                    h0_all[:, tt * D:(tt + 1) * D],
                    prod[:].rearrange("p h k f -> p h f k"),
                    axis=AX.X, op=ALU.add)
                hsq = wp.tile([P, D], f32, tag="hsq")
                nc.scalar.square(hsq[:], h0_all[:, tt * D:(tt + 1) * D])
                nc.tensor.matmul(
                    stats[:, :D], ind_s[:, tt * GPAD:(tt + 1) * GPAD],
                    h0_all[:, tt * D:(tt + 1) * D],
                    start=first_mm, stop=(tt == ntiles - 1))
                nc.tensor.matmul(
                    stats[:, D:], ind_s[:, tt * GPAD:(tt + 1) * GPAD],
                    hsq[:],
                    start=first_mm, stop=(tt == ntiles - 1))
                first_mm = False
                s16base += s["s16"]

            # ---------------- phase D: per-graph P/Q ----------------
            st = pp.tile([GPAD, 2 * D], f32)
            nc.vector.tensor_copy(st[:], stats[:])
            cntinv = gaux_s[:, 0:1]
            bias_o = gaux_s[:, 1:129]
            ms = gaux_s[:, 129:257]
            gnw = gaux_s[:, 257:385]
            gnb = gaux_s[:, 385:513]
            s1 = st[:, :D]
            s2 = st[:, D:]
            m0 = pp.tile([GPAD, D], f32)
            nc.vector.tensor_scalar_mul(m0[:], s1, cntinv)
            mh = pp.tile([GPAD, D], f32)
            nc.vector.tensor_tensor(out=mh[:], in0=m0[:], in1=bias_o,
                                    op=ALU.add)
            t1 = pp.tile([GPAD, D], f32)
            nc.vector.scalar_tensor_tensor(out=t1[:], in0=m0[:], scalar=2.0,
                                           in1=bias_o, op0=ALU.mult,
                                           op1=ALU.add)
            t2 = pp.tile([GPAD, D], f32)
            nc.vector.tensor_tensor(out=t2[:], in0=bias_o, in1=t1[:],
                                    op=ALU.mult)
            e2 = pp.tile([GPAD, D], f32)
            nc.vector.tensor_scalar_mul(e2[:], s2, cntinv)
            nc.vector.tensor_tensor(out=e2[:], in0=e2[:], in1=t2[:],
                                    op=ALU.add)
            c0 = pp.tile([GPAD, D], f32)
            nc.vector.tensor_tensor(out=c0[:], in0=mh[:], in1=ms,
                                    op=ALU.mult)
            t3 = pp.tile([GPAD, D], f32)
            nc.vector.tensor_tensor(out=t3[:], in0=c0[:], in1=mh[:],
                                    op=ALU.mult)
            var = pp.tile([GPAD, D], f32)
            nc.vector.scalar_tensor_tensor(out=var[:], in0=t3[:],
                                           scalar=-2.0, in1=e2[:],
                                           op0=ALU.mult, op1=ALU.add)
            t4 = pp.tile([GPAD, D], f32)
            nc.vector.tensor_tensor(out=t4[:], in0=c0[:], in1=c0[:],
                                    op=ALU.mult)
            nc.vector.tensor_tensor(out=var[:], in0=var[:], in1=t4[:],
                                    op=ALU.add)
            stdv = pp.tile([GPAD, D], f32)
            epsc = pp.tile([GPAD, 1], f32)
            nc.vector.memset(epsc[:], EPS)
            nc.scalar.activation(stdv[:], var[:], ACT.Sqrt, bias=epsc[:],
                                 scale=1.0)
            rstd = pp.tile([GPAD, D], f32)
            nc.vector.reciprocal(rstd[:], stdv[:])
            PQ = pp.tile([GPAD, 2 * D], f32)
            nc.vector.tensor_tensor(out=PQ[:, :D], in0=gnw, in1=rstd[:],
                                    op=ALU.mult)
            t5 = pp.tile([GPAD, D], f32)
            nc.vector.tensor_tensor(out=t5[:], in0=bias_o, in1=c0[:],
                                    op=ALU.subtract)
            nc.vector.tensor_tensor(out=PQ[:, D:], in0=t5[:], in1=PQ[:, :D],
                                    op=ALU.mult)
            nc.vector.tensor_tensor(out=PQ[:, D:], in0=PQ[:, D:], in1=gnb,
                                    op=ALU.add)

            # ---------------- phase E: normalize + relu + out ----------
            for t in range(ntiles):
                indt_t = wp.tile([GPAD, P], f32, tag="indt")
                nc.sync.dma_start(indt_t[:], indT[:, t * P:(t + 1) * P])
                pq = psp.tile([P, 2 * D], f32, tag="pq")
                nc.tensor.matmul(pq[:], indt_t[:], PQ[:],
                                 start=True, stop=True)
                hf = wp.tile([P, D], f32, tag="hf")
                nc.vector.tensor_tensor(out=hf[:],
                                        in0=h0_all[:, t * D:(t + 1) * D],
                                        in1=pq[:, :D], op=ALU.mult)
                nc.vector.tensor_tensor(out=hf[:], in0=hf[:], in1=pq[:, D:],
                                        op=ALU.add)
                ho = wp.tile([P, D], f32, tag="ho")
                nc.scalar.activation(ho[:], hf[:], ACT.Relu)
                nc.sync.dma_start(h_out[t * P:(t + 1) * P, :], ho[:])

    nc.compile()
    return nc


_CACHE = {}


def kernel(node, edge_index, edge_attr, batch_ptr, W_bases, W_comb, b_comb,
           bias_out, gn_weight, gn_bias, gn_mean_scale):
    node = np.asarray(node)
    edge_index = np.asarray(edge_index)
    batch_ptr = np.asarray(batch_ptr)
    cfg = _prep(edge_index, batch_ptr)
    in_maps = _make_inputs(cfg, node, W_bases, W_comb, b_comb, bias_out,
                           gn_weight, gn_bias, gn_mean_scale)

    if os.environ.get("EGC_NUMPY_SIM"):
        return _assemble(cfg, _numpy_sim(cfg, in_maps))

    from concourse.bass_utils import run_bass_kernel_spmd
    key = "prog"
    if key not in _CACHE:
        _CACHE[key] = _build(cfg)
    nc = _CACHE[key]
    res = run_bass_kernel_spmd(nc, in_maps, core_ids=list(range(NCORES)),
                               **_CACHE.get("run_kwargs", {}))
    _CACHE["last_res"] = res
    return _assemble(cfg, [res.results[c]["h"] for c in range(NCORES)])



# revision 26
# speedup vs baseline: 1.0893x; 1.0893x over previous
"""EGConv + GraphNorm + ReLU Trainium2 kernel (8 NeuronCores, SPMD). v2

Strategy (hardcoded for N=100000, E=3200000, D=128, H=8, B=4, A=['sum','max'],
G=64 graphs):
  - Nodes partitioned across 8 cores at graph boundaries (GraphNorm stays
    core-local). Each core owns its dst nodes and their incident edges.
  - Per-edge messages (bases rows, 64 f32 = 256B) gathered via SWDGE
    dma_gather from an allgathered global table. Gather throughput is
    descriptor-COUNT bound (~2.24ns/idx/core at 4 queues), so the design
    minimizes gather indices:
      * 2 source chunks (not 4) using the SIGNED int16 index range: the
        ucode sign-extends idx and only trims TRAILING negatives, so with
        the in_ap base at the chunk center, idx in [-25600, 25599] covers
        a 51200-row chunk. Each call's final slot is kept nonnegative.
      * Self-loops are NOT gathered: phase A keeps each core's own shifted
        bases (bsh = bases + BIG) in SBUF and folds them into the reduce.
  - Table layout is range-major: row = chunk*C2 + core*HALF + (pos%HALF),
    so the allgather splits into 2 collectives (one per chunk) that
    pipeline with phase A; chunk-0 gathers start before collective 1 ends.
    bases_full is a Shared DRAM tensor (fast shared-output AllGather).
    pos HALF-1 of every core is a reserved zero row so each chunk has an
    in-chunk pad row at rel +25599 (nonnegative).
  - Reductions: contiguous pairwise tree adds/maxes on VectorE (no strided
    tensor_reduce), einsum reduced over k by a 3-level pairwise tree.
  - GraphNorm segment stats via indicator matmuls on TensorE (PSUM
    accumulation across all tiles), per-graph affine P/Q, final
    normalize+relu per tile.
"""

import math
import os
import numpy as np

# ---------------- problem constants (hardcoded per spec) ----------------
N = 100000
E = 3200000
D = 128
H = 8
NB = 4          # num bases
FH = 16         # per-head dim
G = 64          # graphs
EPS = 1e-5
P = 128
NCORES = 8
BF = 64         # bases feature dim = NB*FH
BIG = 512.0
GPAD = 16       # padded per-core graph count
COLCAP = 8      # max gather columns per call (8*128 = 1024 idx)
ICHUNK = 2      # tiles per idx-load batch
ACHUNK = 6      # tiles per nodeT-load batch

_F32 = np.float32


def _ceil(a, b):
    return -(-a // b) * b


# ======================================================================
# host preprocessing
# ======================================================================
def _prep(edge_index, batch_ptr):
    counts = np.bincount(batch_ptr, minlength=G).astype(np.int64)
    gcum = np.concatenate([[0], np.cumsum(counts)])  # [G+1]

    # core boundaries at graph boundaries, close to N/8 multiples
    gb = [0]
    for c in range(1, NCORES):
        tgt = N * c / NCORES
        g = int(np.argmin(np.abs(gcum - tgt)))
        g = min(max(g, gb[-1]), G - (NCORES - c))
        gb.append(g)
    gb.append(G)
    node0 = np.array([gcum[gb[c]] for c in range(NCORES)], np.int64)
    ncs = np.array([gcum[gb[c + 1]] - gcum[gb[c]] for c in range(NCORES)],
                   np.int64)
    # n_c + reserved pad (pos HALF-1) + final pad (pos NMAX-1) must fit
    NMAX = _ceil(int(ncs.max()) + 2, 2 * P)
    HALF = NMAX // 2
    C2 = NCORES * HALF
    CENTER = C2 // 2
    assert C2 <= 65536 and CENTER <= 32768
    RESV = HALF - 1           # reserved zero row position (chunk 0)
    ZREL = HALF - 1 + (NCORES - 1) * HALF - CENTER
    # chunk-0 zero row: core7, pos RESV -> row 7*HALF + RESV, rel = ZREL
    # chunk-1 zero row: core7, pos NMAX-1 -> row C2 + 7*HALF + HALF-1,
    #   rel (from center1 = C2 + CENTER) = same ZREL
    assert 0 <= ZREL < 32768
    ntiles = NMAX // P
    HT = HALF // P

    src_g = np.asarray(edge_index[0], np.int64)
    dst_g = np.asarray(edge_index[1], np.int64)
    bounds = np.concatenate([node0, [N]])
    node_core = np.searchsorted(bounds, np.arange(N), side="right") - 1

    dst_core = node_core[dst_g]

    # ---- pass 1: preliminary sort by total in-degree (per core) ----
    cores = []
    for c in range(NCORES):
        n_c = int(ncs[c])
        m = dst_core == c
        es = src_g[m]
        ed = dst_g[m] - node0[c]
        ktot = np.bincount(ed, minlength=n_c)
        rank = np.empty(n_c, np.int64)
        rank[np.argsort(-ktot, kind="stable")] = np.arange(n_c)
        cores.append(dict(n=n_c, node0=int(node0[c]), es=es, ed=ed,
                          rank=rank, g0=gb[c], g1=gb[c + 1]))

    # preliminary chunk of every node (fixed from here on)
    chunk_of = np.empty(N, np.int64)
    for c in range(NCORES):
        cc = cores[c]
        chunk_of[cc["node0"]:cc["node0"] + cc["n"]] = (cc["rank"] >= RESV)

    # ---- pass 2: per-(dst, chunk) counts, refine sort within halves ----
    for c in range(NCORES):
        cc = cores[c]
        n_c = cc["n"]
        ch = chunk_of[cc["es"]]
        kmat = np.bincount(cc["ed"] * 2 + ch, minlength=n_c * 2).reshape(
            n_c, 2)
        kmx = kmat.max(axis=1)
        key = np.lexsort((-kmat[:, 1], -kmat[:, 0], -kmx))
        # stable partition: first the half-0 nodes (rank < RESV) in key
        # order, then half-1, so chunk membership is preserved
        in0 = cc["rank"][key] < RESV
        order = np.concatenate([key[in0], key[~in0]])
        n0 = int(in0.sum())
        pos = np.empty(n_c, np.int64)
        pos[order[:n0]] = np.arange(n0)
        pos[order[n0:]] = HALF + np.arange(n_c - n0)
        assert n0 == RESV
        cc["pos_of"] = pos
        cc["kmat"] = kmat

    # global table row and gather rel of every node
    rel_of = np.empty(N, np.int64)
    for c in range(NCORES):
        cc = cores[c]
        pos = cc["pos_of"]
        rel_of[cc["node0"]:cc["node0"] + cc["n"]] = \
            c * HALF + (pos % HALF) - CENTER
    assert rel_of.min() >= -32768 and rel_of.max() < 32768

    # ---- per-core sorted-space per-chunk CSR ----
    for c in range(NCORES):
        cc = cores[c]
        dpos = cc["pos_of"][cc["ed"]]
        ch = chunk_of[cc["es"]]
        key = dpos * 2 + ch
        eorder = np.argsort(key, kind="stable")
        rel = rel_of[cc["es"]][eorder]
        kflat = np.bincount(key, minlength=NMAX * 2)
        cc["csr_data"] = rel.astype(np.int64)
        cc["csr_ptr"] = np.concatenate([[0], np.cumsum(kflat)])
        cc["ks"] = kflat.reshape(NMAX, 2)

    # shared per-tile per-chunk window widths (max over cores)
    Wct = np.zeros((ntiles, 2), np.int64)
    for c in range(NCORES):
        km = cores[c]["ks"].reshape(ntiles, P, 2)
        Wct = np.maximum(Wct, km.max(axis=1))

    # ---- last-slot feasibility: every call's final idx must be >= 0 ----
    # (ucode trims trailing negatives -> missing writes). dst at p=127 of
    # each tile must be able to place nonneg values (pads are ZREL >= 0)
    # at every call-final column of its (tile, chunk) window.
    for t in range(ntiles):
        for ch in range(2):
            while True:
                wc = int(Wct[t, ch])
                if wc == 0:
                    break
                nfin = -(-wc // COLCAP)
                ok = True
                for c in range(NCORES):
                    cc = cores[c]
                    dp = t * P + P - 1
                    k = int(cc["ks"][dp, ch])
                    p0 = cc["csr_ptr"][dp * 2 + ch]
                    vals = cc["csr_data"][p0:p0 + k]
                    nonneg = int((vals >= 0).sum()) + (wc - k)
                    if nonneg < nfin:
                        ok = False
                        break
                if ok:
                    break
                Wct[t, ch] += 1

    # ---- schedule: per tile, per chunk, calls of <= COLCAP columns ----
    sched = []
    s16tot = 0
    for t in range(ntiles):
        wc = Wct[t].copy()
        wtot = int(wc.sum())
        coff = np.array([0, int(wc[0])])
        calls = []  # (ch, col0_in_msg, ncols, s16off_in_tile)
        s16 = 0
        for ch in range(2):
            done = 0
            while done < int(wc[ch]):
                nc_ = min(COLCAP, int(wc[ch]) - done)
                calls.append((ch, int(coff[ch]) + done, nc_, s16))
                s16 += nc_ * 8
                done += nc_
        sched.append(dict(t=t, wc=wc, wtot=wtot, coff=coff, calls=calls,
                          s16=s16, s16base=s16tot))
        s16tot += s16
    S16TOT = s16tot
    ACCW = max((s["wtot"] + 1) // 2 for s in sched)

    # ---- per-core device input arrays ----
    for c in range(NCORES):
        cc = cores[c]
        data, ptr, ks = cc["csr_data"], cc["csr_ptr"], cc["ks"]
        idx16 = np.zeros((P, S16TOT), np.int16)
        for s in sched:
            t = s["t"]
            wtot = s["wtot"]
            mat = np.full((P, wtot), ZREL, np.int64)
            for ch in range(2):
                wcch = int(s["wc"][ch])
                if wcch == 0:
                    continue
                c0 = int(s["coff"][ch])
                dp = t * P + np.arange(P)
                cnt = ks[dp, ch]
                start = ptr[dp * 2 + ch]
                jj = np.arange(wcch)[None, :]
                gidx = np.minimum(start[:, None] + jj,
                                  max(len(data) - 1, 0))
                vals = data[gidx] if len(data) else np.full(
                    (P, wcch), ZREL, np.int64)
                mat[:, c0:c0 + wcch] = np.where(jj < cnt[:, None], vals,
                                                ZREL)
                # p=127: nonneg values at call-final columns
                k = int(cnt[P - 1])
                p0 = ptr[(t * P + P - 1) * 2 + ch]
                v = np.concatenate([data[p0:p0 + k],
                                    np.full(wcch - k, ZREL, np.int64)])
                fin = np.zeros(wcch, bool)
                done = 0
                while done < wcch:
                    ncl = min(COLCAP, wcch - done)
                    fin[done + ncl - 1] = True
                    done += ncl
                nfin = int(fin.sum())
                vneg = v[v < 0]
                vpos = v[v >= 0]
                assert len(vpos) >= nfin, (t, ch, c)
                row = np.empty(wcch, np.int64)
                row[fin] = vpos[:nfin]
                row[~fin] = np.concatenate([vpos[nfin:], vneg])
                mat[P - 1, c0:c0 + wcch] = row
            for (ch, col0, ncols, s16off) in s["calls"]:
                flat = mat[:, col0:col0 + ncols].T.reshape(-1)
                wrapped = flat.reshape(-1, 16).T.astype(np.int16)
                idx16[:, s["s16base"] + s16off:
                      s["s16base"] + s16off + ncols * 8] = np.tile(
                          wrapped, (8, 1))
        cc["idx16"] = idx16

        real = np.zeros(NMAX, bool)
        real[cc["pos_of"]] = True
        # device folds RAW self bases (tmp = braw + kbig), so kbig cancels
        # only the k_ns BIG shifts of the gathered window slots
        kttl = ks.sum(axis=1)
        cc["kbig"] = (kttl.reshape(ntiles, P).T.astype(_F32)
                      * _F32(-BIG)).astype(_F32)
        cc["shift"] = np.where(real.reshape(ntiles, P).T, _F32(BIG),
                               _F32(0.0)).astype(_F32)

        # graph id per sorted position
        gid = np.full(NMAX, -1, np.int64)
        gnode = np.searchsorted(gcum, cc["node0"] + np.arange(cc["n"]),
                                side="right") - 1 - cc["g0"]
        gid[cc["pos_of"]] = gnode
        ind = np.zeros((P, ntiles * GPAD), _F32)
        indT = np.zeros((GPAD, ntiles * P), _F32)
        for tt in range(ntiles):
            gl = gid[tt * P:(tt + 1) * P]
            valid = gl >= 0
            pidx = np.arange(P)[valid]
            gv = gl[valid]
            ind[pidx, tt * GPAD + gv] = 1.0
            indT[gv, tt * P + pidx] = 1.0
        cc["ind"] = ind
        cc["indT"] = indT
        cc["gid"] = gid

        cnt_loc = counts[cc["g0"]:cc["g1"]].astype(_F32)
        cntinv = np.zeros(GPAD, _F32)
        cntinv[:len(cnt_loc)] = 1.0 / np.maximum(cnt_loc, 1.0)
        cc["cntinv"] = cntinv

    return dict(cores=cores, NMAX=NMAX, HALF=HALF, C2=C2, CENTER=CENTER,
                ntiles=ntiles, HT=HT, sched=sched, S16TOT=S16TOT,
                ACCW=ACCW, node0=node0, ncs=ncs)


def _make_inputs(cfg, node, W_bases, W_comb, b_comb, bias_out, gn_weight,
                 gn_bias, gn_mean_scale):
    node = np.asarray(node, _F32)
    NMAX = cfg["NMAX"]
    wcat = np.concatenate([np.asarray(W_bases, _F32),
                           np.asarray(W_comb, _F32)], axis=1)  # [128,128]
    bcomb = np.asarray(b_comb, _F32).reshape(1, BF)
    gaux = np.zeros((GPAD, 520), _F32)
    gaux[:, 1:129] = np.asarray(bias_out, _F32)[None, :]
    gaux[:, 129:257] = np.asarray(gn_mean_scale, _F32)[None, :]
    gaux[:, 257:385] = np.asarray(gn_weight, _F32)[None, :]
    gaux[:, 385:513] = np.asarray(gn_bias, _F32)[None, :]

    in_maps = []
    for c in range(NCORES):
        cc = cfg["cores"][c]
        nperm = np.zeros((NMAX, D), _F32)
        nperm[cc["pos_of"]] = node[cc["node0"]:cc["node0"] + cc["n"]]
        ga = gaux.copy()
        ga[:, 0] = cc["cntinv"]
        in_maps.append({
            "nodeT": np.ascontiguousarray(nperm.T),        # [128, NMAX]
            "wcat": wcat,
            "bcomb": bcomb,
            "idx": cc["idx16"],                            # [128, S16TOT]
            "kbig": np.ascontiguousarray(cc["kbig"]),      # [128, ntiles]
            "shift": np.ascontiguousarray(cc["shift"]),    # [128, ntiles]
            "ind": np.ascontiguousarray(cc["ind"]),        # [128, nt*16]
            "indT": np.ascontiguousarray(cc["indT"]),      # [16, nt*128]
            "gaux": ga,                                    # [16, 520]
        })
    return in_maps


# ======================================================================
# numpy simulation of the device algorithm (bit-approximate, for testing)
# ======================================================================
def _numpy_sim(cfg, in_maps):
    NMAX, HALF, C2 = cfg["NMAX"], cfg["HALF"], cfg["C2"]
    CENTER, ntiles = cfg["CENTER"], cfg["ntiles"]
    # phase A+B: bases table (range-major layout), per-core comb, bsh
    table = np.zeros((2 * C2, BF), _F32)
    combs, bshs = [], []
    for c in range(NCORES):
        im = in_maps[c]
        full = im["nodeT"].T @ im["wcat"]  # [NMAX, 128]
        shift = im["shift"].T.reshape(-1)  # [NMAX] pos-major
        bsh = full[:, :BF] + shift[:, None]
        comb = full[:, BF:] + im["bcomb"][0][None, :]
        table[0 * C2 + c * HALF: 0 * C2 + (c + 1) * HALF] = bsh[:HALF]
        table[1 * C2 + c * HALF: 1 * C2 + (c + 1) * HALF] = bsh[HALF:]
        combs.append(comb)
        bshs.append(full[:, :BF].copy())  # raw bases (device uses bf16)

    outs = []
    for c in range(NCORES):
        im = in_maps[c]
        h0 = np.zeros((NMAX, D), _F32)
        kbig = im["kbig"].T  # [ntiles, 128]
        for s in cfg["sched"]:
            t, wtot = s["t"], s["wtot"]
            msg = np.zeros((P, wtot, BF), _F32)
            for (ch, col0, ncols, s16off) in s["calls"]:
                blk = im["idx"][:16, s["s16base"] + s16off:
                                s["s16base"] + s16off + ncols * 8]
                f2 = blk.T.reshape(-1)
                vals = f2[:ncols * 128].astype(np.int64)
                rows = table[ch * C2 + CENTER + vals.reshape(ncols, P)]
                msg[:, col0:col0 + ncols, :] = rows.transpose(1, 0, 2)
            braw_t = bshs[c][t * P:(t + 1) * P]
            ssum = msg.sum(axis=1) + braw_t + kbig[t][:, None]
            smax = np.maximum(msg.max(axis=1),
                              braw_t + _F32(BIG)) - _F32(BIG)
            aggcat = np.concatenate([ssum, smax], axis=1)
            comb = combs[c][t * P:(t + 1) * P]  # [128, 64]
            prod = (comb.reshape(P, H, 8, 1) *
                    aggcat.reshape(P, 1, 8, FH))
            h0[t * P:(t + 1) * P] = prod.sum(axis=2).reshape(P, D)
        # graphnorm
        ind = im["ind"].reshape(P, ntiles, GPAD)
        ga = im["gaux"]
        cntinv = ga[:, 0:1]
        bias_o = ga[:, 1:129]
        ms = ga[:, 129:257]
        gnw = ga[:, 257:385]
        gnb = ga[:, 385:513]
        s1 = np.zeros((GPAD, D), _F32)
        s2 = np.zeros((GPAD, D), _F32)
        for tt in range(ntiles):
            ht = h0[tt * P:(tt + 1) * P]
            s1 += ind[:, tt, :].T @ ht
            s2 += ind[:, tt, :].T @ (ht * ht)
        m0 = s1 * cntinv
        mh = m0 + bias_o
        e2 = s2 * cntinv + bias_o * (2 * m0 + bias_o)
        c0 = mh * ms
        var = e2 - 2 * c0 * mh + c0 * c0
        rstd = 1.0 / np.sqrt(var + EPS)
        Pm = gnw * rstd
        Qm = (bias_o - c0) * Pm + gnb
        indT = im["indT"].reshape(GPAD, ntiles, P)
        hfin = np.zeros((NMAX, D), _F32)
        for tt in range(ntiles):
            Pn = indT[:, tt, :].T @ Pm
            Qn = indT[:, tt, :].T @ Qm
            hfin[tt * P:(tt + 1) * P] = np.maximum(
                h0[tt * P:(tt + 1) * P] * Pn + Qn, 0.0)
        outs.append(hfin)
    return outs


def _assemble(cfg, per_core_h):
    out = np.zeros((N, D), _F32)
    for c in range(NCORES):
        cc = cfg["cores"][c]
        out[cc["node0"] + np.arange(cc["n"])] = \
            per_core_h[c][cc["pos_of"]]
    return out


# ======================================================================
# device program
# ======================================================================
def _build(cfg):
    import concourse.bacc as bacc
    import concourse.tile as tile
    from concourse import mybir

    NMAX, HALF, C2 = cfg["NMAX"], cfg["HALF"], cfg["C2"]
    CENTER, ntiles, HT = cfg["CENTER"], cfg["ntiles"], cfg["HT"]
    S16TOT, ACCW = cfg["S16TOT"], cfg["ACCW"]
    f32 = mybir.dt.float32
    bf16 = mybir.dt.bfloat16
    ALU = mybir.AluOpType
    ACT = mybir.ActivationFunctionType

    nc = bacc.Bacc("TRN2", target_bir_lowering=False, debug=False,
                   num_devices=NCORES, num_swdge_queues=4,
                   dynamic_dma_scratch_size=8192)

    nodeT = nc.dram_tensor("nodeT", [P, NMAX], f32, kind="ExternalInput").ap()
    wcat = nc.dram_tensor("wcat", [D, D], f32, kind="ExternalInput").ap()
    bcomb = nc.dram_tensor("bcomb", [1, BF], f32, kind="ExternalInput").ap()
    idx = nc.dram_tensor("idx", [P, S16TOT], mybir.dt.int16,
                         kind="ExternalInput").ap()
    kbig = nc.dram_tensor("kbig", [P, ntiles], f32, kind="ExternalInput").ap()
    shift = nc.dram_tensor("shift", [P, ntiles], f32,
                           kind="ExternalInput").ap()
    ind = nc.dram_tensor("ind", [P, ntiles * GPAD], f32,
                         kind="ExternalInput").ap()
    indT = nc.dram_tensor("indT", [GPAD, ntiles * P], f32,
                          kind="ExternalInput").ap()
    gaux = nc.dram_tensor("gaux", [GPAD, 520], f32, kind="ExternalInput").ap()
    h_out = nc.dram_tensor("h", [NMAX, D], f32, kind="ExternalOutput").ap()
    bases_full_lo = nc.dram_tensor("bases_full_lo", [C2, BF], f32,
                                   addr_space="Shared").ap()
    bases_full_hi = nc.dram_tensor("bases_full_hi", [C2, BF], f32,
                                   addr_space="Shared").ap()
    DBG = bool(os.environ.get("EGC_DEBUG"))
    if DBG:
        wtot0 = cfg["sched"][0]["wtot"]
        dbg_lo = nc.dram_tensor("dbg_lo", [C2, BF], f32,
                                kind="ExternalOutput").ap()
        dbg_msg = nc.dram_tensor("dbg_msg", [P, wtot0 * BF], f32,
                                 kind="ExternalOutput").ap()
        dbg_h0 = nc.dram_tensor("dbg_h0", [P, ntiles * D], f32,
                                kind="ExternalOutput").ap()

    with tile.TileContext(nc) as tc:
        with (
            tc.tile_pool(name="dram", bufs=1, space="DRAM") as dram,
            tc.tile_pool(name="persist", bufs=1) as pp,
            tc.tile_pool(name="work", bufs=3) as wp,
            tc.tile_pool(name="idxp", bufs=2) as ixp,
            tc.tile_pool(name="msgp", bufs=2) as mp,
            tc.tile_pool(name="psum", bufs=2, space="PSUM") as psp,
            tc.tile_pool(name="statps", bufs=1, space="PSUM") as stp,
        ):
            bases_slice_lo = dram.tile([HALF, BF], f32)
            bases_slice_hi = dram.tile([HALF, BF], f32)

            # ---- constants / persistent
            wcat_s = pp.tile([D, D], f32)
            nc.sync.dma_start(wcat_s[:], wcat[:])
            bcomb_s = pp.tile([1, BF], f32)
            nc.sync.dma_start(bcomb_s[:], bcomb[:])
            ones1 = pp.tile([1, P], f32)
            nc.vector.memset(ones1[:], 1.0)
            negbig = pp.tile([P, 1], f32)
            nc.vector.memset(negbig[:], -BIG)
            posbig = pp.tile([P, 1], f32)
            nc.vector.memset(posbig[:], BIG)
            kbig_s = pp.tile([P, ntiles], f32)
            nc.sync.dma_start(kbig_s[:], kbig[:])
            shift_s = pp.tile([P, ntiles], f32)
            nc.sync.dma_start(shift_s[:], shift[:])
            gaux_s = pp.tile([GPAD, 520], f32)
            nc.sync.dma_start(gaux_s[:], gaux[:])

            comb_all = pp.tile([P, ntiles * BF], f32)
            braw_all = pp.tile([P, ntiles * BF], bf16)
            h0_all = pp.tile([P, ntiles * D], f32)
            acc_s = pp.tile([P, ACCW, BF], f32)
            acc_m = pp.tile([P, ACCW, BF], f32)

            # ---------------- phase A: bases + comb ----------------
            for t in range(ntiles):
                if t % ACHUNK == 0:
                    nblk = wp.tile([P, ACHUNK * P], f32, tag="nblk")
                    nb = min(ACHUNK, ntiles - t)
                    nc.sync.dma_start(nblk[:, :nb * P],
                                      nodeT[:, t * P:(t + nb) * P])
                nt = nblk[:, (t % ACHUNK) * P:(t % ACHUNK + 1) * P]
                ps = psp.tile([P, D], f32, tag="psA")
                nc.tensor.matmul(ps[:], nt, wcat_s[:], start=True,
                                 stop=False)
                nc.tensor.matmul(ps[:, BF:], ones1[:], bcomb_s[:],
                                 start=False, stop=True)
                bshw = wp.tile([P, BF], f32, tag="bshw")
                nc.scalar.activation(bshw[:], ps[:, :BF], ACT.Identity,
                                     bias=shift_s[:, t:t + 1], scale=1.0)
                nc.scalar.copy(braw_all[:, t * BF:(t + 1) * BF],
                               ps[:, :BF])
                nc.scalar.copy(comb_all[:, t * BF:(t + 1) * BF],
                               ps[:, BF:])
                if t < HT:
                    nc.sync.dma_start(
                        bases_slice_lo[t * P:(t + 1) * P, :], bshw[:])
                else:
                    nc.sync.dma_start(
                        bases_slice_hi[(t - HT) * P:(t - HT + 1) * P, :],
                        bshw[:])
                # -------- phase B: allgather halves (pipelined) --------
                if t == HT - 1:
                    nc.gpsimd.collective_compute(
                        "AllGather", ALU.bypass,
                        replica_groups=[list(range(NCORES))],
                        ins=[bases_slice_lo.opt()],
                        outs=[bases_full_lo[:]],
                    )
                if t == ntiles - 1:
                    nc.gpsimd.collective_compute(
                        "AllGather", ALU.bypass,
                        replica_groups=[list(range(NCORES))],
                        ins=[bases_slice_hi.opt()],
                        outs=[bases_full_hi[:]],
                    )

            # ---------------- phase C: gather + reduce + einsum ----
            stats = stp.tile([GPAD, 2 * D], f32)
            qrot = 0
            for si, s in enumerate(cfg["sched"]):
                t = s["t"]
                wtot = s["wtot"]
                if si % ICHUNK == 0:
                    gs = cfg["sched"][si:si + ICHUNK]
                    gsz = sum(x["s16"] for x in gs)
                    g0 = s["s16base"]
                    idxt = ixp.tile([P, gsz], mybir.dt.int16, tag="idxt")
                    nc.sync.dma_start(idxt[:], idx[:, g0:g0 + gsz])
                    ibase = g0
                if t % ACHUNK == 0:
                    indblk = wp.tile([P, ACHUNK * GPAD], f32, tag="indblk")
                    nb = min(ACHUNK, ntiles - t)
                    nc.sync.dma_start(indblk[:, :nb * GPAD],
                                      ind[:, t * GPAD:(t + nb) * GPAD])
                msg = mp.tile([P, wtot, BF], f32, tag="msg")
                if DBG and si == 0:
                    nc.sync.dma_start(dbg_lo[:], bases_full_lo[:])
                for (ch, col0, ncols, s16off) in s["calls"]:
                    off = s["s16base"] - ibase + s16off
                    bsrc = bases_full_lo if ch == 0 else bases_full_hi
                    nc.gpsimd.dma_gather(
                        msg[:, col0:col0 + ncols, :],
                        bsrc[CENTER:CENTER + 2, :],
                        idxt[:, off:off + ncols * 8],
                        ncols * P, ncols * P, BF,
                        queue_num=qrot % 4,
                        single_packet=True,
                    )
                    qrot += 1
                if DBG and si == 0:
                    nc.sync.dma_start(
                        dbg_msg[:], msg[:].rearrange("p w f -> p (w f)"))
                braw_t = braw_all[:, t * BF:(t + 1) * BF]
                # contiguous pairwise tree reduce (sum into acc_s, max
                # into acc_m); odd leftovers folded into column 0
                w = wtot
                h = w // 2
                if h == 0:
                    nc.vector.tensor_copy(acc_s[:, 0, :], msg[:, 0, :])
                    nc.vector.tensor_copy(acc_m[:, 0, :], msg[:, 0, :])
                else:
                    nc.vector.tensor_tensor(
                        out=acc_s[:, :h, :], in0=msg[:, :h, :],
                        in1=msg[:, h:2 * h, :], op=ALU.add)
                    nc.vector.tensor_tensor(
                        out=acc_m[:, :h, :], in0=msg[:, :h, :],
                        in1=msg[:, h:2 * h, :], op=ALU.max)
                    if w % 2:
                        nc.vector.tensor_tensor(
                            out=acc_s[:, 0, :], in0=acc_s[:, 0, :],
                            in1=msg[:, 2 * h, :], op=ALU.add)
                        nc.vector.tensor_tensor(
                            out=acc_m[:, 0, :], in0=acc_m[:, 0, :],
                            in1=msg[:, 2 * h, :], op=ALU.max)
                    w = h
                    while w > 1:
                        h = w // 2
                        nc.vector.tensor_tensor(
                            out=acc_s[:, :h, :], in0=acc_s[:, :h, :],
                            in1=acc_s[:, h:2 * h, :], op=ALU.add)
                        nc.vector.tensor_tensor(
                            out=acc_m[:, :h, :], in0=acc_m[:, :h, :],
                            in1=acc_m[:, h:2 * h, :], op=ALU.max)
                        if w % 2:
                            nc.vector.tensor_tensor(
                                out=acc_s[:, 0, :], in0=acc_s[:, 0, :],
                                in1=acc_s[:, 2 * h, :], op=ALU.add)
                            nc.vector.tensor_tensor(
                                out=acc_m[:, 0, :], in0=acc_m[:, 0, :],
                                in1=acc_m[:, 2 * h, :], op=ALU.max)
                        w = h
                # self-loop fold + corrections
                tmp = wp.tile([P, BF], f32, tag="tmp")
                nc.scalar.activation(tmp[:], braw_t, ACT.Identity,
                                     bias=kbig_s[:, t:t + 1], scale=1.0)
                tmp2 = wp.tile([P, BF], f32, tag="tmp2")
                nc.scalar.activation(tmp2[:], braw_t, ACT.Identity,
                                     bias=posbig[:], scale=1.0)
                aggcat = wp.tile([P, 2 * BF], f32, tag="aggcat")
                nc.vector.tensor_tensor(out=aggcat[:, :BF],
                                        in0=acc_s[:, 0, :], in1=tmp[:],
                                        op=ALU.add)
                nc.vector.tensor_tensor(out=aggcat[:, BF:],
                                        in0=acc_m[:, 0, :], in1=tmp2[:],
                                        op=ALU.max)
                nc.scalar.activation(aggcat[:, BF:], aggcat[:, BF:],
                                     ACT.Identity, bias=negbig[:],
                                     scale=1.0)
                # einsum: out[p,h,f] = sum_k comb[p,h,k]*agg[p,k,f]
                prod = wp.tile([P, H, 8, FH], f32, tag="prod")
                cview = comb_all[:, t * BF:(t + 1) * BF].rearrange(
                    "p (h k) -> p h k", h=H)
                nc.vector.tensor_tensor(
                    out=prod[:],
                    in0=cview.to_broadcast([P, H, 8, FH]),
                    in1=aggcat[:].rearrange("p (k f) -> p k f", k=8)
                    [:, None, :, :].broadcast_to([P, H, 8, FH]),
                    op=ALU.mult)
                v1 = wp.tile([P, H, 4, FH], f32, tag="v1")
                nc.vector.tensor_tensor(out=v1[:], in0=prod[:, :, :4, :],
                                        in1=prod[:, :, 4:, :], op=ALU.add)
                nc.vector.tensor_tensor(out=v1[:, :, :2, :],
                                        in0=v1[:, :, :2, :],
                                        in1=v1[:, :, 2:, :], op=ALU.add)
                nc.vector.tensor_tensor(
                    out=h0_all[:, t * D:(t + 1) * D].rearrange(
                        "p (h f) -> p h f", h=H),
                    in0=v1[:, :, 0, :], in1=v1[:, :, 1, :], op=ALU.add)
                hsq = wp.tile([P, D], f32, tag="hsq")
                nc.scalar.square(hsq[:], h0_all[:, t * D:(t + 1) * D])
                iv = indblk[:, (t % ACHUNK) * GPAD:
                            (t % ACHUNK + 1) * GPAD]
                nc.tensor.matmul(
                    stats[:, :D], iv,
                    h0_all[:, t * D:(t + 1) * D],
                    start=(t == 0), stop=(t == ntiles - 1))
                nc.tensor.matmul(
                    stats[:, D:], iv, hsq[:],
                    start=(t == 0), stop=(t == ntiles - 1))

            if DBG:
                nc.sync.dma_start(dbg_h0[:], h0_all[:])

            # ---------------- phase D: per-graph P/Q ----------------
            st = pp.tile([GPAD, 2 * D], f32)
            nc.vector.tensor_copy(st[:], stats[:])
            cntinv = gaux_s[:, 0:1]
            bias_o = gaux_s[:, 1:129]
            ms = gaux_s[:, 129:257]
            gnw = gaux_s[:, 257:385]
            gnb = gaux_s[:, 385:513]
            s1 = st[:, :D]
            s2 = st[:, D:]
            m0 = pp.tile([GPAD, D], f32)
            nc.vector.tensor_scalar_mul(m0[:], s1, cntinv)
            mh = pp.tile([GPAD, D], f32)
            nc.vector.tensor_tensor(out=mh[:], in0=m0[:], in1=bias_o,
                                    op=ALU.add)
            t1 = pp.tile([GPAD, D], f32)
            nc.vector.scalar_tensor_tensor(out=t1[:], in0=m0[:], scalar=2.0,
                                           in1=bias_o, op0=ALU.mult,
                                           op1=ALU.add)
            t2 = pp.tile([GPAD, D], f32)
            nc.vector.tensor_tensor(out=t2[:], in0=bias_o, in1=t1[:],
                                    op=ALU.mult)
            e2 = pp.tile([GPAD, D], f32)
            nc.vector.tensor_scalar_mul(e2[:], s2, cntinv)
            nc.vector.tensor_tensor(out=e2[:], in0=e2[:], in1=t2[:],
                                    op=ALU.add)
            c0 = pp.tile([GPAD, D], f32)
            nc.vector.tensor_tensor(out=c0[:], in0=mh[:], in1=ms,
                                    op=ALU.mult)
            t3 = pp.tile([GPAD, D], f32)
            nc.vector.tensor_tensor(out=t3[:], in0=c0[:], in1=mh[:],
                                    op=ALU.mult)
            var = pp.tile([GPAD, D], f32)
            nc.vector.scalar_tensor_tensor(out=var[:], in0=t3[:],
                                           scalar=-2.0, in1=e2[:],
                                           op0=ALU.mult, op1=ALU.add)
            t4 = pp.tile([GPAD, D], f32)
            nc.vector.tensor_tensor(out=t4[:], in0=c0[:], in1=c0[:],
                                    op=ALU.mult)
            nc.vector.tensor_tensor(out=var[:], in0=var[:], in1=t4[:],
                                    op=ALU.add)
            stdv = pp.tile([GPAD, D], f32)
            epsc = pp.tile([GPAD, 1], f32)
            nc.vector.memset(epsc[:], EPS)
            nc.scalar.activation(stdv[:], var[:], ACT.Sqrt, bias=epsc[:],
                                 scale=1.0)
            rstd = pp.tile([GPAD, D], f32)
            nc.vector.reciprocal(rstd[:], stdv[:])
            PQ = pp.tile([GPAD, 2 * D], f32)
            nc.vector.tensor_tensor(out=PQ[:, :D], in0=gnw, in1=rstd[:],
                                    op=ALU.mult)
            t5 = pp.tile([GPAD, D], f32)
            nc.vector.tensor_tensor(out=t5[:], in0=bias_o, in1=c0[:],
                                    op=ALU.subtract)
            nc.vector.tensor_tensor(out=PQ[:, D:], in0=t5[:], in1=PQ[:, :D],
                                    op=ALU.mult)
            nc.vector.tensor_tensor(out=PQ[:, D:], in0=PQ[:, D:], in1=gnb,
                                    op=ALU.add)

            # ---------------- phase E: normalize + relu + out ----------
            for t in range(ntiles):
                if t % ACHUNK == 0:
                    itblk = wp.tile([GPAD, ACHUNK * P], f32, tag="itblk")
                    nb = min(ACHUNK, ntiles - t)
                    nc.sync.dma_start(itblk[:, :nb * P],
                                      indT[:, t * P:(t + nb) * P])
                pq = psp.tile([P, 2 * D], f32, tag="pq")
                nc.tensor.matmul(
                    pq[:], itblk[:, (t % ACHUNK) * P:(t % ACHUNK + 1) * P],
                    PQ[:], start=True, stop=True)
                hf = wp.tile([P, D], f32, tag="hf")
                nc.vector.tensor_tensor(out=hf[:],
                                        in0=h0_all[:, t * D:(t + 1) * D],
                                        in1=pq[:, :D], op=ALU.mult)
                nc.vector.tensor_tensor(out=hf[:], in0=hf[:], in1=pq[:, D:],
                                        op=ALU.add)
                ho = wp.tile([P, D], f32, tag="ho")
                nc.scalar.activation(ho[:], hf[:], ACT.Relu)
                nc.sync.dma_start(h_out[t * P:(t + 1) * P, :], ho[:])

    nc.compile()
    return nc


_CACHE = {}


def kernel(node, edge_index, edge_attr, batch_ptr, W_bases, W_comb, b_comb,
           bias_out, gn_weight, gn_bias, gn_mean_scale):
    node = np.asarray(node)
    edge_index = np.asarray(edge_index)
    batch_ptr = np.asarray(batch_ptr)
    cfg = _prep(edge_index, batch_ptr)
    in_maps = _make_inputs(cfg, node, W_bases, W_comb, b_comb, bias_out,
                           gn_weight, gn_bias, gn_mean_scale)

    if os.environ.get("EGC_NUMPY_SIM"):
        return _assemble(cfg, _numpy_sim(cfg, in_maps))

    from concourse.bass_utils import run_bass_kernel_spmd
    key = "prog"
    if key not in _CACHE:
        _CACHE[key] = _build(cfg)
    nc = _CACHE[key]
    res = run_bass_kernel_spmd(nc, in_maps, core_ids=list(range(NCORES)),
                               **_CACHE.get("run_kwargs", {}))
    _CACHE["last_res"] = res
    return _assemble(cfg, [res.results[c]["h"] for c in range(NCORES)])


# revision 33
# speedup vs baseline: 1.1303x; 1.0376x over previous
"""EGConv + GraphNorm + ReLU Trainium2 kernel (8 NeuronCores, SPMD). v2

Strategy (hardcoded for N=100000, E=3200000, D=128, H=8, B=4, A=['sum','max'],
G=64 graphs):
  - Nodes partitioned across 8 cores at graph boundaries (GraphNorm stays
    core-local). Each core owns its dst nodes and their incident edges.
  - Per-edge messages (bases rows, 64 f32 = 256B) gathered via SWDGE
    dma_gather from an allgathered global table. Gather throughput is
    descriptor-COUNT bound (~2.24ns/idx/core at 4 queues), so the design
    minimizes gather indices:
      * 2 source chunks (not 4) using the SIGNED int16 index range: the
        ucode sign-extends idx and only trims TRAILING negatives, so with
        the in_ap base at the chunk center, idx in [-25600, 25599] covers
        a 51200-row chunk. Each call's final slot is kept nonnegative.
      * Self-loops are NOT gathered: phase A keeps each core's own shifted
        bases (bsh = bases + BIG) in SBUF and folds them into the reduce.
  - Table layout is range-major: row = chunk*C2 + core*HALF + (pos%HALF),
    so the allgather splits into 2 collectives (one per chunk) that
    pipeline with phase A; chunk-0 gathers start before collective 1 ends.
    bases_full is a Shared DRAM tensor (fast shared-output AllGather).
    pos HALF-1 of every core is a reserved zero row so each chunk has an
    in-chunk pad row at rel +25599 (nonnegative).
  - Reductions: contiguous pairwise tree adds/maxes on VectorE (no strided
    tensor_reduce), einsum reduced over k by a 3-level pairwise tree.
  - GraphNorm segment stats via indicator matmuls on TensorE (PSUM
    accumulation across all tiles), per-graph affine P/Q, final
    normalize+relu per tile.
"""

import math
import os
import numpy as np

# ---------------- problem constants (hardcoded per spec) ----------------
N = 100000
E = 3200000
D = 128
H = 8
NB = 4          # num bases
FH = 16         # per-head dim
G = 64          # graphs
EPS = 1e-5
P = 128
NCORES = 8
BF = 64         # bases feature dim = NB*FH
BIG = 512.0
GPAD = 16       # padded per-core graph count
COLCAP = 8      # max gather columns per call (8*128 = 1024 idx)
ICHUNK = 2      # tiles per idx-load batch
ACHUNK = 6      # tiles per nodeT-load batch

_F32 = np.float32


def _ceil(a, b):
    return -(-a // b) * b


# ======================================================================
# host preprocessing
# ======================================================================
def _prep(edge_index, batch_ptr):
    counts = np.bincount(batch_ptr, minlength=G).astype(np.int64)
    gcum = np.concatenate([[0], np.cumsum(counts)])  # [G+1]

    # core boundaries at graph boundaries, close to N/8 multiples
    gb = [0]
    for c in range(1, NCORES):
        tgt = N * c / NCORES
        g = int(np.argmin(np.abs(gcum - tgt)))
        g = min(max(g, gb[-1]), G - (NCORES - c))
        gb.append(g)
    gb.append(G)
    node0 = np.array([gcum[gb[c]] for c in range(NCORES)], np.int64)
    ncs = np.array([gcum[gb[c + 1]] - gcum[gb[c]] for c in range(NCORES)],
                   np.int64)
    # n_c + reserved pad (pos HALF-1) + final pad (pos NMAX-1) must fit
    NMAX = _ceil(int(ncs.max()) + 2, 2 * P)
    HALF = NMAX // 2
    C2 = NCORES * HALF
    CENTER = C2 // 2
    assert C2 <= 65536 and CENTER <= 32768
    RESV = HALF - 1           # reserved zero row position (chunk 0)
    ZREL = HALF - 1 + (NCORES - 1) * HALF - CENTER
    # chunk-0 zero row: core7, pos RESV -> row 7*HALF + RESV, rel = ZREL
    # chunk-1 zero row: core7, pos NMAX-1 -> row C2 + 7*HALF + HALF-1,
    #   rel (from center1 = C2 + CENTER) = same ZREL
    assert 0 <= ZREL < 32768
    ntiles = NMAX // P
    HT = HALF // P

    src_g = np.asarray(edge_index[0], np.int64)
    dst_g = np.asarray(edge_index[1], np.int64)
    bounds = np.concatenate([node0, [N]])
    node_core = np.searchsorted(bounds, np.arange(N), side="right") - 1

    dst_core = node_core[dst_g]

    # ---- pass 1: preliminary sort by total in-degree (per core) ----
    cores = []
    for c in range(NCORES):
        n_c = int(ncs[c])
        m = dst_core == c
        es = src_g[m]
        ed = dst_g[m] - node0[c]
        ktot = np.bincount(ed, minlength=n_c)
        rank = np.empty(n_c, np.int64)
        rank[np.argsort(-ktot, kind="stable")] = np.arange(n_c)
        cores.append(dict(n=n_c, node0=int(node0[c]), es=es, ed=ed,
                          rank=rank, g0=gb[c], g1=gb[c + 1]))

    # preliminary chunk of every node (fixed from here on)
    chunk_of = np.empty(N, np.int64)
    for c in range(NCORES):
        cc = cores[c]
        chunk_of[cc["node0"]:cc["node0"] + cc["n"]] = (cc["rank"] >= RESV)

    # ---- pass 2: per-(dst, chunk) counts, refine sort within halves ----
    for c in range(NCORES):
        cc = cores[c]
        n_c = cc["n"]
        ch = chunk_of[cc["es"]]
        kmat = np.bincount(cc["ed"] * 2 + ch, minlength=n_c * 2).reshape(
            n_c, 2)
        kmx = kmat.max(axis=1)
        key = np.lexsort((-kmat[:, 1], -kmat[:, 0], -kmx))
        # stable partition: first the half-0 nodes (rank < RESV) in key
        # order, then half-1, so chunk membership is preserved
        in0 = cc["rank"][key] < RESV
        order = np.concatenate([key[in0], key[~in0]])
        n0 = int(in0.sum())
        pos = np.empty(n_c, np.int64)
        pos[order[:n0]] = np.arange(n0)
        pos[order[n0:]] = HALF + np.arange(n_c - n0)
        assert n0 == RESV
        cc["pos_of"] = pos
        cc["kmat"] = kmat

    # global table row and gather rel of every node
    rel_of = np.empty(N, np.int64)
    for c in range(NCORES):
        cc = cores[c]
        pos = cc["pos_of"]
        rel_of[cc["node0"]:cc["node0"] + cc["n"]] = \
            c * HALF + (pos % HALF) - CENTER
    assert rel_of.min() >= -32768 and rel_of.max() < 32768

    # ---- per-core sorted-space per-chunk CSR ----
    for c in range(NCORES):
        cc = cores[c]
        dpos = cc["pos_of"][cc["ed"]]
        ch = chunk_of[cc["es"]]
        key = dpos * 2 + ch
        eorder = np.argsort(key, kind="stable")
        rel = rel_of[cc["es"]][eorder]
        kflat = np.bincount(key, minlength=NMAX * 2)
        cc["csr_data"] = rel.astype(np.int64)
        cc["csr_ptr"] = np.concatenate([[0], np.cumsum(kflat)])
        cc["ks"] = kflat.reshape(NMAX, 2)

    # shared per-tile per-chunk window widths (max over cores)
    Wct = np.zeros((ntiles, 2), np.int64)
    for c in range(NCORES):
        km = cores[c]["ks"].reshape(ntiles, P, 2)
        Wct = np.maximum(Wct, km.max(axis=1))

    # ---- last-slot feasibility: every call's final idx must be >= 0 ----
    # (ucode trims trailing negatives -> missing writes). dst at p=127 of
    # each tile must be able to place nonneg values (pads are ZREL >= 0)
    # at every call-final column of its (tile, chunk) window.
    for t in range(ntiles):
        for ch in range(2):
            while True:
                wc = int(Wct[t, ch])
                if wc == 0:
                    break
                nfin = -(-wc // COLCAP)
                ok = True
                for c in range(NCORES):
                    cc = cores[c]
                    dp = t * P + P - 1
                    k = int(cc["ks"][dp, ch])
                    p0 = cc["csr_ptr"][dp * 2 + ch]
                    vals = cc["csr_data"][p0:p0 + k]
                    nonneg = int((vals >= 0).sum()) + (wc - k)
                    if nonneg < nfin:
                        ok = False
                        break
                if ok:
                    break
                Wct[t, ch] += 1

    # ---- schedule: per tile, per chunk, calls of <= COLCAP columns ----
    sched = []
    s16tot = 0
    for t in range(ntiles):
        wc = Wct[t].copy()
        wtot = int(wc.sum())
        coff = np.array([0, int(wc[0])])
        calls = []  # (ch, col0_in_msg, ncols, s16off_in_tile)
        s16 = 0
        for ch in range(2):
            done = 0
            while done < int(wc[ch]):
                nc_ = min(COLCAP, int(wc[ch]) - done)
                calls.append((ch, int(coff[ch]) + done, nc_, s16))
                s16 += nc_ * 8
                done += nc_
        sched.append(dict(t=t, wc=wc, wtot=wtot, coff=coff, calls=calls,
                          s16=s16, s16base=s16tot))
        s16tot += s16
    S16TOT = s16tot

    def _redplan(w):
        # mirror of the device reduce: level-1 pairs (+leftover copy),
        # level-2 pairs (+leftover copy), self column appended, strided
        # tail reduce over w2+1 columns
        h1 = w // 2
        w1 = (h1 + (w % 2)) if h1 > 0 else 1
        h2 = w1 // 2
        w2 = (h2 + (w1 % 2)) if h2 >= 2 else w1
        return max(w1, w2 + 1)
    ACCW = max(_redplan(s["wtot"]) for s in sched)

    # ---- per-core device input arrays ----
    for c in range(NCORES):
        cc = cores[c]
        data, ptr, ks = cc["csr_data"], cc["csr_ptr"], cc["ks"]
        idx16 = np.zeros((P, S16TOT), np.int16)
        for s in sched:
            t = s["t"]
            wtot = s["wtot"]
            mat = np.full((P, wtot), ZREL, np.int64)
            for ch in range(2):
                wcch = int(s["wc"][ch])
                if wcch == 0:
                    continue
                c0 = int(s["coff"][ch])
                dp = t * P + np.arange(P)
                cnt = ks[dp, ch]
                start = ptr[dp * 2 + ch]
                jj = np.arange(wcch)[None, :]
                gidx = np.minimum(start[:, None] + jj,
                                  max(len(data) - 1, 0))
                vals = data[gidx] if len(data) else np.full(
                    (P, wcch), ZREL, np.int64)
                mat[:, c0:c0 + wcch] = np.where(jj < cnt[:, None], vals,
                                                ZREL)
                # p=127: nonneg values at call-final columns
                k = int(cnt[P - 1])
                p0 = ptr[(t * P + P - 1) * 2 + ch]
                v = np.concatenate([data[p0:p0 + k],
                                    np.full(wcch - k, ZREL, np.int64)])
                fin = np.zeros(wcch, bool)
                done = 0
                while done < wcch:
                    ncl = min(COLCAP, wcch - done)
                    fin[done + ncl - 1] = True
                    done += ncl
                nfin = int(fin.sum())
                vneg = v[v < 0]
                vpos = v[v >= 0]
                assert len(vpos) >= nfin, (t, ch, c)
                row = np.empty(wcch, np.int64)
                row[fin] = vpos[:nfin]
                row[~fin] = np.concatenate([vpos[nfin:], vneg])
                mat[P - 1, c0:c0 + wcch] = row
            for (ch, col0, ncols, s16off) in s["calls"]:
                flat = mat[:, col0:col0 + ncols].T.reshape(-1)
                wrapped = flat.reshape(-1, 16).T.astype(np.int16)
                idx16[:, s["s16base"] + s16off:
                      s["s16base"] + s16off + ncols * 8] = np.tile(
                          wrapped, (8, 1))
        cc["idx16"] = idx16

        real = np.zeros(NMAX, bool)
        real[cc["pos_of"]] = True
        # device folds RAW self bases (tmp = braw + kbig), so kbig cancels
        # only the k_ns BIG shifts of the gathered window slots
        kttl = ks.sum(axis=1)
        cc["kbig"] = (kttl.reshape(ntiles, P).T.astype(_F32)
                      * _F32(-BIG)).astype(_F32)
        cc["shift"] = np.where(real.reshape(ntiles, P).T, _F32(BIG),
                               _F32(0.0)).astype(_F32)

        # graph id per sorted position
        gid = np.full(NMAX, -1, np.int64)
        gnode = np.searchsorted(gcum, cc["node0"] + np.arange(cc["n"]),
                                side="right") - 1 - cc["g0"]
        gid[cc["pos_of"]] = gnode
        ind = np.zeros((P, ntiles * GPAD), _F32)
        indT = np.zeros((GPAD, ntiles * P), _F32)
        for tt in range(ntiles):
            gl = gid[tt * P:(tt + 1) * P]
            valid = gl >= 0
            pidx = np.arange(P)[valid]
            gv = gl[valid]
            ind[pidx, tt * GPAD + gv] = 1.0
            indT[gv, tt * P + pidx] = 1.0
        cc["ind"] = ind
        cc["indT"] = indT
        cc["gid"] = gid

        cnt_loc = counts[cc["g0"]:cc["g1"]].astype(_F32)
        cntinv = np.zeros(GPAD, _F32)
        cntinv[:len(cnt_loc)] = 1.0 / np.maximum(cnt_loc, 1.0)
        cc["cntinv"] = cntinv

    return dict(cores=cores, NMAX=NMAX, HALF=HALF, C2=C2, CENTER=CENTER,
                ntiles=ntiles, HT=HT, sched=sched, S16TOT=S16TOT,
                ACCW=ACCW, node0=node0, ncs=ncs)


def _make_inputs(cfg, node, W_bases, W_comb, b_comb, bias_out, gn_weight,
                 gn_bias, gn_mean_scale):
    node = np.asarray(node, _F32)
    NMAX = cfg["NMAX"]
    wcat = np.concatenate([np.asarray(W_bases, _F32),
                           np.asarray(W_comb, _F32)], axis=1)  # [128,128]
    bcomb = np.asarray(b_comb, _F32).reshape(1, BF)
    gaux = np.zeros((GPAD, 520), _F32)
    gaux[:, 1:129] = np.asarray(bias_out, _F32)[None, :]
    gaux[:, 129:257] = np.asarray(gn_mean_scale, _F32)[None, :]
    gaux[:, 257:385] = np.asarray(gn_weight, _F32)[None, :]
    gaux[:, 385:513] = np.asarray(gn_bias, _F32)[None, :]

    in_maps = []
    for c in range(NCORES):
        cc = cfg["cores"][c]
        nperm = np.zeros((NMAX, D), _F32)
        nperm[cc["pos_of"]] = node[cc["node0"]:cc["node0"] + cc["n"]]
        ga = gaux.copy()
        ga[:, 0] = cc["cntinv"]
        in_maps.append({
            "nodeT": np.ascontiguousarray(nperm.T),        # [128, NMAX]
            "wcat": wcat,
            "bcomb": bcomb,
            "idx": cc["idx16"],                            # [128, S16TOT]
            "kbig": np.ascontiguousarray(cc["kbig"]),      # [128, ntiles]
            "shift": np.ascontiguousarray(cc["shift"]),    # [128, ntiles]
            "ind": np.ascontiguousarray(cc["ind"]),        # [128, nt*16]
            "indT": np.ascontiguousarray(cc["indT"]),      # [16, nt*128]
            "gaux": ga,                                    # [16, 520]
        })
    return in_maps


# ======================================================================
# numpy simulation of the device algorithm (bit-approximate, for testing)
# ======================================================================
def _numpy_sim(cfg, in_maps):
    NMAX, HALF, C2 = cfg["NMAX"], cfg["HALF"], cfg["C2"]
    CENTER, ntiles = cfg["CENTER"], cfg["ntiles"]
    # phase A+B: bases table (range-major layout), per-core comb, bsh
    table = np.zeros((2 * C2, BF), _F32)
    combs, bshs = [], []
    for c in range(NCORES):
        im = in_maps[c]
        full = im["nodeT"].T @ im["wcat"]  # [NMAX, 128]
        shift = im["shift"].T.reshape(-1)  # [NMAX] pos-major
        bsh = full[:, :BF] + shift[:, None]
        comb = full[:, BF:] + im["bcomb"][0][None, :]
        table[0 * C2 + c * HALF: 0 * C2 + (c + 1) * HALF] = bsh[:HALF]
        table[1 * C2 + c * HALF: 1 * C2 + (c + 1) * HALF] = bsh[HALF:]
        combs.append(comb)
        bshs.append(full[:, :BF].copy())  # raw bases (device uses bf16)

    outs = []
    for c in range(NCORES):
        im = in_maps[c]
        h0 = np.zeros((NMAX, D), _F32)
        kbig = im["kbig"].T  # [ntiles, 128]
        for s in cfg["sched"]:
            t, wtot = s["t"], s["wtot"]
            msg = np.zeros((P, wtot, BF), _F32)
            for (ch, col0, ncols, s16off) in s["calls"]:
                blk = im["idx"][:16, s["s16base"] + s16off:
                                s["s16base"] + s16off + ncols * 8]
                f2 = blk.T.reshape(-1)
                vals = f2[:ncols * 128].astype(np.int64)
                rows = table[ch * C2 + CENTER + vals.reshape(ncols, P)]
                msg[:, col0:col0 + ncols, :] = rows.transpose(1, 0, 2)
            braw_t = bshs[c][t * P:(t + 1) * P]
            ssum = msg.sum(axis=1) + braw_t + kbig[t][:, None]
            smax = np.maximum(msg.max(axis=1),
                              braw_t + _F32(BIG)) - _F32(BIG)
            aggcat = np.concatenate([ssum, smax], axis=1)
            comb = combs[c][t * P:(t + 1) * P]  # [128, 64]
            prod = (comb.reshape(P, H, 8, 1) *
                    aggcat.reshape(P, 1, 8, FH))
            h0[t * P:(t + 1) * P] = prod.sum(axis=2).reshape(P, D)
        # graphnorm
        ind = im["ind"].reshape(P, ntiles, GPAD)
        ga = im["gaux"]
        cntinv = ga[:, 0:1]
        bias_o = ga[:, 1:129]
        ms = ga[:, 129:257]
        gnw = ga[:, 257:385]
        gnb = ga[:, 385:513]
        s1 = np.zeros((GPAD, D), _F32)
        s2 = np.zeros((GPAD, D), _F32)
        for tt in range(ntiles):
            ht = h0[tt * P:(tt + 1) * P]
            s1 += ind[:, tt, :].T @ ht
            s2 += ind[:, tt, :].T @ (ht * ht)
        m0 = s1 * cntinv
        mh = m0 + bias_o
        e2 = s2 * cntinv + bias_o * (2 * m0 + bias_o)
        c0 = mh * ms
        var = e2 - 2 * c0 * mh + c0 * c0
        rstd = 1.0 / np.sqrt(var + EPS)
        Pm = gnw * rstd
        Qm = (bias_o - c0) * Pm + gnb
        indT = im["indT"].reshape(GPAD, ntiles, P)
        hfin = np.zeros((NMAX, D), _F32)
        for tt in range(ntiles):
            Pn = indT[:, tt, :].T @ Pm
            Qn = indT[:, tt, :].T @ Qm
            hfin[tt * P:(tt + 1) * P] = np.maximum(
                h0[tt * P:(tt + 1) * P] * Pn + Qn, 0.0)
        outs.append(hfin)
    return outs


def _assemble(cfg, per_core_h):
    out = np.zeros((N, D), _F32)
    for c in range(NCORES):
        cc = cfg["cores"][c]
        out[cc["node0"] + np.arange(cc["n"])] = \
            per_core_h[c][cc["pos_of"]]
    return out


# ======================================================================
# device program
# ======================================================================
def _build(cfg):
    import concourse.bacc as bacc
    import concourse.tile as tile
    from concourse import mybir

    NMAX, HALF, C2 = cfg["NMAX"], cfg["HALF"], cfg["C2"]
    CENTER, ntiles, HT = cfg["CENTER"], cfg["ntiles"], cfg["HT"]
    S16TOT, ACCW = cfg["S16TOT"], cfg["ACCW"]
    f32 = mybir.dt.float32
    bf16 = mybir.dt.bfloat16
    ALU = mybir.AluOpType
    ACT = mybir.ActivationFunctionType
    AX = mybir.AxisListType

    nc = bacc.Bacc("TRN2", target_bir_lowering=False, debug=False,
                   num_devices=NCORES, num_swdge_queues=4,
                   dynamic_dma_scratch_size=8192)

    nodeT = nc.dram_tensor("nodeT", [P, NMAX], f32, kind="ExternalInput").ap()
    wcat = nc.dram_tensor("wcat", [D, D], f32, kind="ExternalInput").ap()
    bcomb = nc.dram_tensor("bcomb", [1, BF], f32, kind="ExternalInput").ap()
    idx = nc.dram_tensor("idx", [P, S16TOT], mybir.dt.int16,
                         kind="ExternalInput").ap()
    kbig = nc.dram_tensor("kbig", [P, ntiles], f32, kind="ExternalInput").ap()
    shift = nc.dram_tensor("shift", [P, ntiles], f32,
                           kind="ExternalInput").ap()
    ind = nc.dram_tensor("ind", [P, ntiles * GPAD], f32,
                         kind="ExternalInput").ap()
    indT = nc.dram_tensor("indT", [GPAD, ntiles * P], f32,
                          kind="ExternalInput").ap()
    gaux = nc.dram_tensor("gaux", [GPAD, 520], f32, kind="ExternalInput").ap()
    h_out = nc.dram_tensor("h", [NMAX, D], f32, kind="ExternalOutput").ap()
    bases_full_lo = nc.dram_tensor("bases_full_lo", [C2, BF], f32,
                                   addr_space="Shared").ap()
    bases_full_hi = nc.dram_tensor("bases_full_hi", [C2, BF], f32,
                                   addr_space="Shared").ap()
    DBG = bool(os.environ.get("EGC_DEBUG"))
    if DBG:
        wtot0 = cfg["sched"][0]["wtot"]
        dbg_lo = nc.dram_tensor("dbg_lo", [C2, BF], f32,
                                kind="ExternalOutput").ap()
        dbg_msg = nc.dram_tensor("dbg_msg", [P, wtot0 * BF], f32,
                                 kind="ExternalOutput").ap()
        dbg_h0 = nc.dram_tensor("dbg_h0", [P, ntiles * D], f32,
                                kind="ExternalOutput").ap()

    with tile.TileContext(nc) as tc:
        with (
            tc.tile_pool(name="dram", bufs=1, space="DRAM") as dram,
            tc.tile_pool(name="persist", bufs=1) as pp,
            tc.tile_pool(name="work", bufs=3) as wp,
            tc.tile_pool(name="idxp", bufs=2) as ixp,
            tc.tile_pool(name="msgp", bufs=2) as mp,
            tc.tile_pool(name="psum", bufs=2, space="PSUM") as psp,
            tc.tile_pool(name="statps", bufs=1, space="PSUM") as stp,
        ):
            bases_slice_lo = dram.tile([HALF, BF], f32)
            bases_slice_hi = dram.tile([HALF, BF], f32)

            # ---- constants / persistent
            wcat_s = pp.tile([D, D], f32)
            nc.sync.dma_start(wcat_s[:], wcat[:])
            bcomb_s = pp.tile([1, BF], f32)
            nc.sync.dma_start(bcomb_s[:], bcomb[:])
            ones1 = pp.tile([1, P], f32)
            nc.vector.memset(ones1[:], 1.0)
            negbig = pp.tile([P, 1], f32)
            nc.vector.memset(negbig[:], -BIG)
            posbig = pp.tile([P, 1], f32)
            nc.vector.memset(posbig[:], BIG)
            kbig_s = pp.tile([P, ntiles], f32)
            nc.sync.dma_start(kbig_s[:], kbig[:])
            shift_s = pp.tile([P, ntiles], f32)
            nc.sync.dma_start(shift_s[:], shift[:])
            gaux_s = pp.tile([GPAD, 520], f32)
            nc.sync.dma_start(gaux_s[:], gaux[:])

            comb_all = pp.tile([P, ntiles * BF], f32)
            braw_all = pp.tile([P, ntiles * BF], bf16)
            h0_all = pp.tile([P, ntiles * D], f32)
            acc_s = pp.tile([P, ACCW, BF], f32)
            acc_m = pp.tile([P, ACCW, BF], f32)

            # ---------------- phase A: bases + comb ----------------
            for t in range(ntiles):
                if t % ACHUNK == 0:
                    nblk = wp.tile([P, ACHUNK * P], f32, tag="nblk")
                    nb = min(ACHUNK, ntiles - t)
                    nc.sync.dma_start(nblk[:, :nb * P],
                                      nodeT[:, t * P:(t + nb) * P])
                nt = nblk[:, (t % ACHUNK) * P:(t % ACHUNK + 1) * P]
                ps = psp.tile([P, D], f32, tag="psA")
                nc.tensor.matmul(ps[:], nt, wcat_s[:], start=True,
                                 stop=False)
                nc.tensor.matmul(ps[:, BF:], ones1[:], bcomb_s[:],
                                 start=False, stop=True)
                bshw = wp.tile([P, BF], f32, tag="bshw")
                nc.scalar.activation(bshw[:], ps[:, :BF], ACT.Identity,
                                     bias=shift_s[:, t:t + 1], scale=1.0)
                nc.scalar.copy(braw_all[:, t * BF:(t + 1) * BF],
                               ps[:, :BF])
                nc.scalar.copy(comb_all[:, t * BF:(t + 1) * BF],
                               ps[:, BF:])
                if t < HT:
                    nc.sync.dma_start(
                        bases_slice_lo[t * P:(t + 1) * P, :], bshw[:])
                else:
                    nc.sync.dma_start(
                        bases_slice_hi[(t - HT) * P:(t - HT + 1) * P, :],
                        bshw[:])
                # -------- phase B: allgather halves (pipelined) --------
                if t == HT - 1:
                    nc.gpsimd.collective_compute(
                        "AllGather", ALU.bypass,
                        replica_groups=[list(range(NCORES))],
                        ins=[bases_slice_lo.opt()],
                        outs=[bases_full_lo[:]],
                    )
                if t == ntiles - 1:
                    nc.gpsimd.collective_compute(
                        "AllGather", ALU.bypass,
                        replica_groups=[list(range(NCORES))],
                        ins=[bases_slice_hi.opt()],
                        outs=[bases_full_hi[:]],
                    )

            # ---------------- phase C: gather + reduce + einsum ----
            stats = stp.tile([GPAD, 2 * D], f32)
            qrot = 0
            for si, s in enumerate(cfg["sched"]):
                t = s["t"]
                wtot = s["wtot"]
                if si % ICHUNK == 0:
                    gs = cfg["sched"][si:si + ICHUNK]
                    gsz = sum(x["s16"] for x in gs)
                    g0 = s["s16base"]
                    idxt = ixp.tile([P, gsz], mybir.dt.int16, tag="idxt")
                    nc.sync.dma_start(idxt[:], idx[:, g0:g0 + gsz])
                    ibase = g0
                if t % ACHUNK == 0:
                    indblk = wp.tile([P, ACHUNK * GPAD], f32, tag="indblk")
                    nb = min(ACHUNK, ntiles - t)
                    nc.sync.dma_start(indblk[:, :nb * GPAD],
                                      ind[:, t * GPAD:(t + nb) * GPAD])
                msg = mp.tile([P, wtot, BF], f32, tag="msg")
                if DBG and si == 0:
                    nc.sync.dma_start(dbg_lo[:], bases_full_lo[:])
                for (ch, col0, ncols, s16off) in s["calls"]:
                    off = s["s16base"] - ibase + s16off
                    bsrc = bases_full_lo if ch == 0 else bases_full_hi
                    nc.gpsimd.dma_gather(
                        msg[:, col0:col0 + ncols, :],
                        bsrc[CENTER:CENTER + 2, :],
                        idxt[:, off:off + ncols * 8],
                        ncols * P, ncols * P, BF,
                        queue_num=qrot % 4,
                        single_packet=True,
                    )
                    qrot += 1
                if DBG and si == 0:
                    nc.sync.dma_start(
                        dbg_msg[:], msg[:].rearrange("p w f -> p (w f)"))
                braw_t = braw_all[:, t * BF:(t + 1) * BF]
                # level-1 pairs msg -> acc, level-2 pairs in acc, self
                # column injected on ScalarE, strided tail reduce
                w = wtot
                h1 = w // 2
                if h1 == 0:
                    nc.vector.tensor_copy(acc_s[:, 0, :], msg[:, 0, :])
                    nc.vector.tensor_copy(acc_m[:, 0, :], msg[:, 0, :])
                    w2 = 1
                else:
                    nc.vector.tensor_tensor(
                        out=acc_s[:, :h1, :], in0=msg[:, :h1, :],
                        in1=msg[:, h1:2 * h1, :], op=ALU.add)
                    nc.vector.tensor_tensor(
                        out=acc_m[:, :h1, :], in0=msg[:, :h1, :],
                        in1=msg[:, h1:2 * h1, :], op=ALU.max)
                    w1 = h1
                    if w % 2:
                        nc.vector.tensor_copy(acc_s[:, h1, :],
                                              msg[:, w - 1, :])
                        nc.vector.tensor_copy(acc_m[:, h1, :],
                                              msg[:, w - 1, :])
                        w1 = h1 + 1
                    h2 = w1 // 2
                    if h2 >= 2:
                        nc.vector.tensor_tensor(
                            out=acc_s[:, :h2, :], in0=acc_s[:, :h2, :],
                            in1=acc_s[:, h2:2 * h2, :], op=ALU.add)
                        nc.vector.tensor_tensor(
                            out=acc_m[:, :h2, :], in0=acc_m[:, :h2, :],
                            in1=acc_m[:, h2:2 * h2, :], op=ALU.max)
                        w2 = h2
                        if w1 % 2:
                            nc.vector.tensor_copy(acc_s[:, h2, :],
                                                  acc_s[:, w1 - 1, :])
                            nc.vector.tensor_copy(acc_m[:, h2, :],
                                                  acc_m[:, w1 - 1, :])
                            w2 = h2 + 1
                    else:
                        w2 = w1
                # inject self column: sum gets braw+kbig, max braw+BIG
                nc.scalar.activation(acc_s[:, w2, :], braw_t,
                                     ACT.Identity,
                                     bias=kbig_s[:, t:t + 1], scale=1.0)
                nc.scalar.activation(acc_m[:, w2, :], braw_t,
                                     ACT.Identity, bias=posbig[:],
                                     scale=1.0)
                aggcat = wp.tile([P, 2 * BF], f32, tag="aggcat")
                nc.vector.tensor_reduce(
                    aggcat[:, :BF],
                    acc_s[:, :w2 + 1, :].rearrange("p w f -> p f w"),
                    axis=AX.X, op=ALU.add)
                nc.vector.tensor_reduce(
                    aggcat[:, BF:],
                    acc_m[:, :w2 + 1, :].rearrange("p w f -> p f w"),
                    axis=AX.X, op=ALU.max)
                nc.scalar.activation(aggcat[:, BF:], aggcat[:, BF:],
                                     ACT.Identity, bias=negbig[:],
                                     scale=1.0)
                # einsum: out[p,h,f] = sum_k comb[p,h,k]*agg[p,k,f]
                prod = wp.tile([P, H, 8, FH], f32, tag="prod")
                cview = comb_all[:, t * BF:(t + 1) * BF].rearrange(
                    "p (h k) -> p h k", h=H)
                nc.vector.tensor_tensor(
                    out=prod[:],
                    in0=cview.to_broadcast([P, H, 8, FH]),
                    in1=aggcat[:].rearrange("p (k f) -> p k f", k=8)
                    [:, None, :, :].broadcast_to([P, H, 8, FH]),
                    op=ALU.mult)
                v1 = wp.tile([P, H, 4, FH], f32, tag="v1")
                nc.vector.tensor_tensor(out=v1[:], in0=prod[:, :, :4, :],
                                        in1=prod[:, :, 4:, :], op=ALU.add)
                nc.vector.tensor_tensor(out=v1[:, :, :2, :],
                                        in0=v1[:, :, :2, :],
                                        in1=v1[:, :, 2:, :], op=ALU.add)
                nc.vector.tensor_tensor(
                    out=h0_all[:, t * D:(t + 1) * D].rearrange(
                        "p (h f) -> p h f", h=H),
                    in0=v1[:, :, 0, :], in1=v1[:, :, 1, :], op=ALU.add)
                hsq = wp.tile([P, D], f32, tag="hsq")
                nc.scalar.square(hsq[:], h0_all[:, t * D:(t + 1) * D])
                iv = indblk[:, (t % ACHUNK) * GPAD:
                            (t % ACHUNK + 1) * GPAD]
                nc.tensor.matmul(
                    stats[:, :D], iv,
                    h0_all[:, t * D:(t + 1) * D],
                    start=(t == 0), stop=(t == ntiles - 1))
                nc.tensor.matmul(
                    stats[:, D:], iv, hsq[:],
                    start=(t == 0), stop=(t == ntiles - 1))

            if DBG:
                nc.sync.dma_start(dbg_h0[:], h0_all[:])

            # ---------------- phase D: per-graph P/Q ----------------
            st = pp.tile([GPAD, 2 * D], f32)
            nc.vector.tensor_copy(st[:], stats[:])
            cntinv = gaux_s[:, 0:1]
            bias_o = gaux_s[:, 1:129]
            ms = gaux_s[:, 129:257]
            gnw = gaux_s[:, 257:385]
            gnb = gaux_s[:, 385:513]
            s1 = st[:, :D]
            s2 = st[:, D:]
            m0 = pp.tile([GPAD, D], f32)
            nc.vector.tensor_scalar_mul(m0[:], s1, cntinv)
            mh = pp.tile([GPAD, D], f32)
            nc.vector.tensor_tensor(out=mh[:], in0=m0[:], in1=bias_o,
                                    op=ALU.add)
            t1 = pp.tile([GPAD, D], f32)
            nc.vector.scalar_tensor_tensor(out=t1[:], in0=m0[:], scalar=2.0,
                                           in1=bias_o, op0=ALU.mult,
                                           op1=ALU.add)
            t2 = pp.tile([GPAD, D], f32)
            nc.vector.tensor_tensor(out=t2[:], in0=bias_o, in1=t1[:],
                                    op=ALU.mult)
            e2 = pp.tile([GPAD, D], f32)
            nc.vector.tensor_scalar_mul(e2[:], s2, cntinv)
            nc.vector.tensor_tensor(out=e2[:], in0=e2[:], in1=t2[:],
                                    op=ALU.add)
            c0 = pp.tile([GPAD, D], f32)
            nc.vector.tensor_tensor(out=c0[:], in0=mh[:], in1=ms,
                                    op=ALU.mult)
            t3 = pp.tile([GPAD, D], f32)
            nc.vector.tensor_tensor(out=t3[:], in0=c0[:], in1=mh[:],
                                    op=ALU.mult)
            var = pp.tile([GPAD, D], f32)
            nc.vector.scalar_tensor_tensor(out=var[:], in0=t3[:],
                                           scalar=-2.0, in1=e2[:],
                                           op0=ALU.mult, op1=ALU.add)
            t4 = pp.tile([GPAD, D], f32)
            nc.vector.tensor_tensor(out=t4[:], in0=c0[:], in1=c0[:],
                                    op=ALU.mult)
            nc.vector.tensor_tensor(out=var[:], in0=var[:], in1=t4[:],
                                    op=ALU.add)
            stdv = pp.tile([GPAD, D], f32)
            epsc = pp.tile([GPAD, 1], f32)
            nc.vector.memset(epsc[:], EPS)
            nc.scalar.activation(stdv[:], var[:], ACT.Sqrt, bias=epsc[:],
                                 scale=1.0)
            rstd = pp.tile([GPAD, D], f32)
            nc.vector.reciprocal(rstd[:], stdv[:])
            PQ = pp.tile([GPAD, 2 * D], f32)
            nc.vector.tensor_tensor(out=PQ[:, :D], in0=gnw, in1=rstd[:],
                                    op=ALU.mult)
            t5 = pp.tile([GPAD, D], f32)
            nc.vector.tensor_tensor(out=t5[:], in0=bias_o, in1=c0[:],
                                    op=ALU.subtract)
            nc.vector.tensor_tensor(out=PQ[:, D:], in0=t5[:], in1=PQ[:, :D],
                                    op=ALU.mult)
            nc.vector.tensor_tensor(out=PQ[:, D:], in0=PQ[:, D:], in1=gnb,
                                    op=ALU.add)

            # ---------------- phase E: normalize + relu + out ----------
            EB = 4
            for g in range(0, ntiles, EB):
                nb = min(EB, ntiles - g)
                itblk = wp.tile([GPAD, EB * P], f32, tag="itblk")
                nc.sync.dma_start(itblk[:, :nb * P],
                                  indT[:, g * P:(g + nb) * P])
                pq = psp.tile([P, EB, 2 * D], f32, tag="pq")
                for j in range(nb):
                    nc.tensor.matmul(
                        pq[:, j, :], itblk[:, j * P:(j + 1) * P],
                        PQ[:], start=True, stop=True)
                hf = wp.tile([P, EB, D], f32, tag="hf")
                h0v = h0_all[:, g * D:(g + nb) * D].rearrange(
                    "p (t d) -> p t d", t=nb)
                nc.vector.tensor_tensor(out=hf[:, :nb, :], in0=h0v,
                                        in1=pq[:, :nb, :D], op=ALU.mult)
                nc.vector.tensor_tensor(out=hf[:, :nb, :],
                                        in0=hf[:, :nb, :],
                                        in1=pq[:, :nb, D:], op=ALU.add)
                ho = wp.tile([P, EB, D], f32, tag="ho")
                nc.scalar.activation(ho[:, :nb, :], hf[:, :nb, :],
                                     ACT.Relu)
                for j in range(nb):
                    t = g + j
                    nc.sync.dma_start(h_out[t * P:(t + 1) * P, :],
                                      ho[:, j, :])

    nc.compile()
    return nc


_CACHE = {}


def kernel(node, edge_index, edge_attr, batch_ptr, W_bases, W_comb, b_comb,
           bias_out, gn_weight, gn_bias, gn_mean_scale):
    node = np.asarray(node)
    edge_index = np.asarray(edge_index)
    batch_ptr = np.asarray(batch_ptr)
    cfg = _prep(edge_index, batch_ptr)
    in_maps = _make_inputs(cfg, node, W_bases, W_comb, b_comb, bias_out,
                           gn_weight, gn_bias, gn_mean_scale)

    if os.environ.get("EGC_NUMPY_SIM"):
        return _assemble(cfg, _numpy_sim(cfg, in_maps))

    from concourse.bass_utils import run_bass_kernel_spmd
    key = "prog"
    if key not in _CACHE:
        _CACHE[key] = _build(cfg)
    nc = _CACHE[key]
    res = run_bass_kernel_spmd(nc, in_maps, core_ids=list(range(NCORES)),
                               **_CACHE.get("run_kwargs", {}))
    _CACHE["last_res"] = res
    return _assemble(cfg, [res.results[c]["h"] for c in range(NCORES)])
